# revision 34
# baseline (speedup 1.0000x reference)
"""Trainium2 Bass kernel for nn_AdaptiveGraphConvLayer (graph multi-head attention).

Reference computation:
    mask = dense additive edge mask from edge_index (symmetric + self loops)
    per head h: q,k,v projections of x; scores = q @ k.T / 16 + mask; softmax
    o_h = attn @ v_h; head_out_h = o_h @ Wo_h.T + bo_h
    out = concat_h(head_out) @ Wp.T + bp;  LayerNorm(out) * gamma + beta
    (N=4096 nodes, D=256, H=4 heads, E=131072 edges; ~80 GFLOP)

Measured: ~227 us HW exec on 8 NeuronCores, rel err 3.2e-3 (bf16 matmuls,
fp32 accumulate/softmax/LayerNorm).

Device strategy (kernel(): node-parallel, zero collectives):
  - Core c owns query rows [c*512, (c+1)*512) for ALL 4 heads; k/v
    projections are recomputed per core.  On this setup a measured
    collective costs ~45-60 us (floor-dominated), more than the ~55 us of
    replicated projection matmuls it could remove, so the comm-free layout
    wins (a head-parallel + ReduceScatter variant, _build2/_run2, measured
    equal at best).
  - Algebraic fold: out = sum_h attn_h @ v'_h + bias_tot with
        v'_h = x @ (Wv_h^T (Wp_h Wo_h)^T)   (host-precomputed weight)
    which eliminates the per-head out-proj and final projection entirely.
  - scoresT blocks [kv=128, q=512] = kT-slices^T @ qT; exp on ACT
    (scale=1/16, no max-subtract needed: |scores| < ~1 and every row has a
    self loop); mask applied multiplicatively on DVE; softmax denominator
    via ones-columns appended to v' (o_ext[:, D] = row sum), normalized
    with a per-partition reciprocal.  o-matmuls run one kv-chunk behind the
    exp/mask pipeline so the PE never stalls.
  - Edge mask: host reshards edge_index into per-core dense {0,1} bf16
    stripes in SBUF layout (indirect-DMA scatter on real HW honors only one
    offset per partition per instruction, so an on-device build would cost
    ~260 serial SWDGE instructions ~ 300 us; host resharding keeps all
    FLOPs and all on-chip traffic on device).
  - bf16 everywhere on the PE (fast weight load; fp32r needs pre-rounded
    operands and loads weights 2x slower), fp32 PSUM accumulate, fp32
    softmax/normalize/LayerNorm.  psum->sbuf casts split across ACT/DVE.
  - Prologue: 40 dummy warmup matmuls keep the PE HAM clock-gate at 8/8
    through the input-DMA window; inputs land via few merged strided DMAs
    (sync-queue issue costs ~0.65 us per DMA instruction).
  - Tail: fused Square+accum_out variance, Sqrt table preloaded, affine
    LN ops elided when gamma/beta/bias are trivial for the given inputs.
"""

import numpy as np

N_FULL = 4096
D = 256
H = 4
N_CORES = 8
EPS = 1e-5
P = 128  # partitions


def _build(N, QW, mask_dt_name="bfloat16", mode="f32r",
           triv_bias=False, triv_gamma=False, triv_beta=False):
    """Build + compile the SPMD Bass graph (identical on all cores)."""
    import concourse.bacc as bacc
    import concourse.tile as tile
    import concourse.bass as bass
    from concourse import mybir

    f32 = mybir.dt.float32
    i32 = mybir.dt.int32
    mask_dt = getattr(mybir.dt, mask_dt_name)
    cdt = {"f32r": mybir.dt.float32r, "bf16": mybir.dt.bfloat16,
           "f32": f32}[mode]
    Exp = mybir.ActivationFunctionType.Exp
    Copy = mybir.ActivationFunctionType.Copy
    Sqrt = mybir.ActivationFunctionType.Sqrt
    AX = mybir.AxisListType.X
    MUL = mybir.AluOpType.mult
    KV = N // P            # kv chunks of 128
    QS = QW // P           # q slices of 128 within this core's window
    NB = N // 512          # 512-wide node blocks (kT projection)
    D1 = D + 2             # v' + ones columns (padded even for fp32r)

    def mc(ap):
        return ap

    nc = bacc.Bacc("TRN2", target_bir_lowering=False, debug=False,
                   num_devices=N_CORES)

    xT_d = nc.dram_tensor("xT", [D, N], cdt, kind="ExternalInput").ap()
    xq_d = nc.dram_tensor("xq", [D, QW], cdt, kind="ExternalInput").ap()
    wq_d = nc.dram_tensor("wq", [H, D, D], cdt, kind="ExternalInput").ap()
    wk_d = nc.dram_tensor("wk", [H, D, D], cdt, kind="ExternalInput").ap()
    wv_d = nc.dram_tensor("wv", [H, D, D], cdt, kind="ExternalInput").ap()
    gam_d = nc.dram_tensor("gamma_b", [P, D], f32, kind="ExternalInput").ap()
    bet_d = nc.dram_tensor("beta_b", [P, D], f32, kind="ExternalInput").ap()
    bia_d = nc.dram_tensor("bias_b", [P, D], f32, kind="ExternalInput").ap()
    mal_d = nc.dram_tensor("mall", [P, (N // P) * QW], mask_dt,
                           kind="ExternalInput").ap()
    out_d = nc.dram_tensor("out", [QW, D], f32, kind="ExternalOutput").ap()

    with tile.TileContext(nc) as tc:
        with (
            tc.tile_pool(name="const", bufs=1) as cp,
            tc.tile_pool(name="khead", bufs=2) as kp,
            tc.tile_pool(name="vhead", bufs=2) as vp,
            tc.tile_pool(name="maskp", bufs=1) as mp,
            tc.tile_pool(name="qhead", bufs=2) as qp,
            tc.tile_pool(name="work", bufs=4) as wp,
            tc.tile_pool(name="accs", bufs=1) as ac,
            tc.tile_pool(name="ln", bufs=2) as lp,
            tc.tile_pool(name="psA", bufs=4, space="PSUM") as psA,
            tc.tile_pool(name="psO", bufs=1, space="PSUM") as psO,
            tc.tile_pool(name="dram", bufs=1, space="DRAM") as dp,
        ):
            # ---------- PE warmup: dummy matmuls on uninitialized SBUF so
            # the HAM clock-gate reaches K=8/8 while input DMAs stream in.
            wu = cp.tile([P, 640], mybir.dt.bfloat16, tag="wu")
            nc.vector.memset(wu[:], 0.125)
            wups = psA.tile([P, 512], f32, tag="ps", name="wups")
            for _ in range(40):
                nc.tensor.matmul(wups[:], lhsT=wu[:, :P], rhs=wu[:, P:P + 512],
                                 start=True, stop=True)

            # ---------- load inputs into SBUF ----------
            # DMA queue is FIFO: land the q-projection inputs first so the
            # first real matmuls start as early as possible.
            xq = cp.tile([P, 2 * QW], cdt, tag="xq")
            nc.sync.dma_start(out=xq[:].rearrange("p (i q) -> p i q", q=QW),
                              in_=xq_d[:].rearrange("(i p) q -> p i q", p=P))
            wq = cp.tile([P, H * 2 * D], cdt, tag="wq")
            wk = cp.tile([P, H * 2 * D], cdt, tag="wk")
            wv = cp.tile([P, H * 2 * D], cdt, tag="wv")
            for wsb, wd in ((wq, wq_d), (wk, wk_d), (wv, wv_d)):
                nc.sync.dma_start(
                    out=wsb[:].rearrange("p (h i d) -> p h i d", h=H, i=2),
                    in_=wd[:].rearrange("h (i p) d -> p h i d", p=P))
            xT = cp.tile([P, 2 * N], cdt, tag="xT")
            NQ = N // 4
            for q4 in range(4):
                nc.sync.dma_start(
                    out=xT[:].rearrange("p (i n) -> p i n", n=N)
                        [:, :, q4 * NQ:(q4 + 1) * NQ],
                    in_=xT_d[:].rearrange("(i p) n -> p i n", p=P)
                        [:, :, q4 * NQ:(q4 + 1) * NQ])
            gam = cp.tile([P, D], f32, tag="gam")
            bet = cp.tile([P, D], f32, tag="bet")
            bia = cp.tile([P, D], f32, tag="bia")
            nc.sync.dma_start(out=gam[:], in_=gam_d[:])
            nc.sync.dma_start(out=bet[:], in_=bet_d[:])
            nc.sync.dma_start(out=bia[:], in_=bia_d[:])
            epsc = cp.tile([P, 1], f32, tag="epsc")
            nc.gpsimd.memset(epsc[:], EPS)
            eps2 = cp.tile([P, 1], f32, tag="eps2")
            nc.gpsimd.memset(eps2[:], float(D) * float(D) * EPS)
            onescol = cp.tile([P, 2 * KV], f32, tag="onescol")
            nc.gpsimd.memset(onescol[:], 1.0)
            sqwarm = cp.tile([P, 1], f32, tag="sqwarm")
            nc.scalar.activation(sqwarm[:], epsc[:], Sqrt, bias=epsc[:])

            # ---------- edge-mask stripe (host-sharded input) to SBUF ----
            # quarters: issued after inputs on the same queue; attention
            # chunk c waits only for its quarter
            Mall = mp.tile([P, KV * QW], mask_dt, tag="mask")
            MQ = KV // 4
            for q4 in range(4):
                nc.sync.dma_start(
                    out=Mall[:, q4 * MQ * QW:(q4 + 1) * MQ * QW],
                    in_=mal_d[:, q4 * MQ * QW:(q4 + 1) * MQ * QW])

            # ---------- per-head compute ----------
            acc = [ac.tile([P, D], f32, tag=f"acc{s}", name=f"acc{s}")
                   for s in range(QS)]

            def make_proj(h):
                """Allocate head-h tiles; return (tiles, emit-thunks).

                Each thunk emits one PSUM matmul pair + its psum->sbuf copy;
                thunks are interleaved into the previous head's attention so
                the copies spread over a window where DVE/ACT have slack."""
                qT = qp.tile([P, 2 * QW], cdt, tag="qT", name=f"qT{h}")
                kT = kp.tile([P, 2 * N], cdt, tag="kT", name=f"kT{h}")
                vE = vp.tile([P, KV * D1], cdt, tag="vE", name=f"vE{h}")
                ops = []
                eng = [0]

                def qT_pair(j):
                    ps = psA.tile([P, 512], f32, tag="ps", name=f"q{h}_{j}")
                    for i in range(2):
                        w = (h * 2 + i) * D + j * P
                        nc.tensor.matmul(ps[:, :QW], lhsT=wq[:, w:w + P],
                                         rhs=xq[:, i * QW:(i + 1) * QW],
                                         start=(i == 0), stop=(i == 1))
                    nc.vector.tensor_copy(qT[:, j * QW:(j + 1) * QW],
                                          ps[:, :QW])

                def kT_pair(b, j):
                    ps = psA.tile([P, 512], f32, tag="ps", name=f"k{h}_{b}_{j}")
                    for i in range(2):
                        w = (h * 2 + i) * D + j * P
                        nc.tensor.matmul(
                            ps[:], lhsT=wk[:, w:w + P],
                            rhs=xT[:, i * N + b * 512:i * N + (b + 1) * 512],
                            start=(i == 0), stop=(i == 1))
                    dst = kT[:, j * N + b * 512:j * N + (b + 1) * 512]
                    if eng[0] % 2 == 0:
                        nc.scalar.copy(dst, ps[:])
                    else:
                        nc.vector.tensor_copy(dst, ps[:])
                    eng[0] += 1

                def vE_pair(c):
                    ps = psA.tile([P, 512], f32, tag="ps", name=f"v{h}_{c}")
                    for i in range(2):
                        nc.tensor.matmul(
                            ps[:, :D],
                            lhsT=xT[:, i * N + c * P:i * N + c * P + P],
                            rhs=wv[:, (h * 2 + i) * D:(h * 2 + i + 1) * D],
                            start=(i == 0), stop=(i == 1))
                    nc.vector.tensor_copy(vE[:, c * D1:c * D1 + D], ps[:, :D])

                def ones_cols():
                    nc.vector.tensor_copy(
                        vE[:].rearrange("p (c e) -> p c e", e=D1)[:, :, D:D + 2],
                        onescol[:].rearrange("p (c e) -> p c e", e=2))

                for j in range(2):
                    ops.append(lambda j=j: qT_pair(j))
                for b in range(NB):
                    for j in range(2):
                        ops.append(lambda b=b, j=j: kT_pair(b, j))
                for c in range(KV):
                    ops.append(lambda c=c: vE_pair(c))
                ops.append(ones_cols)
                return (qT, kT, vE), ops

            cur, ops0 = make_proj(0)
            for op in ops0:
                op()

            for h in range(H):
                qT, kT, vE = cur
                if h + 1 < H:
                    nxt, pend = make_proj(h + 1)
                else:
                    nxt, pend = None, []
                per_chunk = -(-len(pend) // (KV - 4)) if pend else 0

                oPS = [psO.tile([P, D1], f32, tag=f"oPS{s}", name=f"oPS{s}")
                       for s in range(QS)]

                def o_mms(c, et):
                    for s in range(QS):
                        nc.tensor.matmul(oPS[s][:],
                                         lhsT=et[:, s * P:(s + 1) * P],
                                         rhs=vE[:, c * D1:(c + 1) * D1],
                                         start=(c == 0), stop=(c == KV - 1))

                prev = None
                for c in range(KV):
                    sc = psA.tile([P, 512], f32, tag="ps")
                    for j in range(2):
                        nc.tensor.matmul(sc[:, :QW],
                                         lhsT=kT[:, j * N + c * P:j * N + c * P + P],
                                         rhs=qT[:, j * QW:(j + 1) * QW],
                                         start=(j == 0), stop=(j == 1))
                    et = wp.tile([P, QW], cdt, tag="et")
                    nc.scalar.activation(et[:], sc[:, :QW], Exp, scale=1.0 / 16.0)
                    nc.vector.tensor_mul(et[:], et[:], Mall[:, c * QW:(c + 1) * QW])
                    if prev is not None:
                        o_mms(*prev)
                    prev = (c, et)
                    if c >= 4:
                        for _ in range(per_chunk):
                            if pend:
                                pend.pop(0)()
                o_mms(*prev)
                while pend:
                    pend.pop(0)()

                for s in range(QS):
                    rec = lp.tile([P, 1], f32, tag="rec")
                    nc.vector.reciprocal(rec[:], oPS[s][:, D:D + 1])
                    if h == 0:
                        nc.vector.tensor_scalar_mul(acc[s][:], oPS[s][:, 0:D], rec[:])
                    else:
                        tmp = lp.tile([P, D], f32, tag="tmp")
                        nc.vector.tensor_scalar_mul(tmp[:], oPS[s][:, 0:D], rec[:])
                        nc.vector.tensor_add(acc[s][:], acc[s][:], tmp[:])
                cur = nxt

            # ---------- bias + LayerNorm + store ----------
            inv_d = 1.0 / D
            Square = mybir.ActivationFunctionType.Square
            for s in range(QS):
                t = acc[s]
                if not triv_bias:
                    nc.vector.tensor_add(t[:], t[:], bia[:])
                musum = lp.tile([P, 1], f32, tag="musum")
                nc.vector.reduce_sum(musum[:], t[:], axis=AX)
                # LN is scale-invariant: center as D*t - sum(t), compensate in
                # the sqrt (scale 1/D, bias D^2*eps) -- one op fewer per slice
                xc = lp.tile([P, D], f32, tag="xc")
                nc.vector.tensor_scalar(out=xc[:], in0=t[:], scalar1=float(D),
                                        scalar2=musum[:],
                                        op0=MUL, op1=mybir.AluOpType.subtract)
                sq = lp.tile([P, D], f32, tag="sq")
                vs = lp.tile([P, 1], f32, tag="vs")
                nc.scalar.activation(sq[:], xc[:], Square, accum_out=vs[:])
                sd = lp.tile([P, 1], f32, tag="sd")
                nc.scalar.activation(sd[:], vs[:], Sqrt, bias=eps2[:], scale=inv_d)
                rs = lp.tile([P, 1], f32, tag="rs")
                nc.vector.reciprocal(rs[:], sd[:])
                og = lp.tile([P, D], f32, tag="og")
                if triv_gamma:
                    nc.vector.tensor_scalar_mul(og[:], xc[:], rs[:])
                else:
                    nc.vector.scalar_tensor_tensor(og[:], in0=xc[:],
                                                   scalar=rs[:], in1=gam[:],
                                                   op0=MUL, op1=MUL)
                if triv_beta:
                    nc.sync.dma_start(out=out_d[s * P:(s + 1) * P, :], in_=og[:])
                else:
                    oo = lp.tile([P, D], f32, tag="oo")
                    nc.vector.tensor_add(oo[:], og[:], bet[:])
                    nc.sync.dma_start(out=out_d[s * P:(s + 1) * P, :], in_=oo[:])

    nc.compile()
    return nc


def _prep_host(inputs, N, QW):
    """Host-side input resharding: transposes, folded weights, mask offsets."""
    x = np.ascontiguousarray(np.asarray(inputs["x"], dtype=np.float32))
    ei = np.asarray(inputs["edge_index"]).astype(np.int64)
    Wq = np.asarray(inputs["Wq"], dtype=np.float64)
    Wk = np.asarray(inputs["Wk"], dtype=np.float64)
    Wv = np.asarray(inputs["Wv"], dtype=np.float64)
    Wo = np.asarray(inputs["Wo"], dtype=np.float64)
    Wp = np.asarray(inputs["Wp"], dtype=np.float64)
    bq = np.asarray(inputs["bq"], dtype=np.float64)
    bk = np.asarray(inputs["bk"], dtype=np.float64)
    bv = np.asarray(inputs["bv"], dtype=np.float64)
    bo = np.asarray(inputs["bo"], dtype=np.float64)
    bp = np.asarray(inputs["bp"], dtype=np.float64)
    gamma = np.asarray(inputs["gamma"], dtype=np.float32)
    beta = np.asarray(inputs["beta"], dtype=np.float32)

    assert not bq.any() and not bk.any(), \
        "nonzero q/k biases not wired in the device graph"

    xT = np.ascontiguousarray(x.T)                       # [D, N]
    wq_h = np.ascontiguousarray(
        np.stack([Wq[h].T for h in range(H)]).astype(np.float32))
    wk_h = np.ascontiguousarray(
        np.stack([Wk[h].T for h in range(H)]).astype(np.float32))
    # folded v' weight and total bias
    wv_l, bias_tot = [], bp.copy()
    for h in range(H):
        Wp_h = Wp[:, h * D:(h + 1) * D]                  # [f, e']
        G = Wo[h].T @ Wp_h.T                             # [e, f]
        wv_l.append(Wv[h].T @ G)                         # [d, f]
        bias_tot = bias_tot + bo[h] @ Wp_h.T + bv[h] @ G
    wv_h = np.ascontiguousarray(np.stack(wv_l).astype(np.float32))

    gam_b = np.ascontiguousarray(np.broadcast_to(gamma, (P, D)).astype(np.float32))
    bet_b = np.ascontiguousarray(np.broadcast_to(beta, (P, D)).astype(np.float32))
    bia_b = np.ascontiguousarray(
        np.broadcast_to(bias_tot.astype(np.float32), (P, D)))

    # mask stripes per core, pre-arranged to the SBUF layout
    # mall[p, c*QW + q] = adjacency[c*P + p, q0 + q]  (kv-major, symmetric+diag)
    import ml_dtypes
    adj = np.zeros((N, N), dtype=np.uint8)
    r, c = ei[0], ei[1]
    adj[r, c] = 1
    adj[c, r] = 1
    adj[np.arange(N), np.arange(N)] = 1
    KV = N // P
    malls = []
    for core in range(N_CORES):
        q0 = core * QW
        stripe = adj[:, q0:q0 + QW]                      # [N(kv), QW]
        m = stripe.reshape(KV, P, QW).transpose(1, 0, 2).reshape(P, KV * QW)
        malls.append(np.ascontiguousarray(m.astype(ml_dtypes.bfloat16)))
    return xT, wq_h, wk_h, wv_h, gam_b, bet_b, bia_b, malls


def _run(inputs, trace=False, mask_dt_name="bfloat16", mode="f32r",
         tmpdir=None):
    from concourse.bass_utils import run_bass_kernel_spmd
    from concourse.bass_interp import get_hw_module

    N = int(np.asarray(inputs["x"]).shape[0])
    QW = N // N_CORES
    (xT, wq_h, wk_h, wv_h, gam_b, bet_b, bia_b, malls) = \
        _prep_host(inputs, N, QW)

    if mode == "bf16":
        import ml_dtypes
        hdt = ml_dtypes.bfloat16
        xT = xT.astype(hdt)
        wq_h, wk_h, wv_h = (a.astype(hdt) for a in (wq_h, wk_h, wv_h))
    elif mode == "f32r":
        # fp32r operands must be pre-rounded (RNE dropping 12 mantissa bits);
        # matches walrus fp32_to_fp32r.
        def _r(a):
            b = a.view(np.uint32).astype(np.uint64)
            rb = (b + 0x7FF + ((b >> 12) & 1)) & np.uint64(0xFFFFF000)
            return rb.astype(np.uint32).view(np.float32)
        xT = _r(xT)
        wq_h, wk_h, wv_h = _r(wq_h), _r(wk_h), _r(wv_h)
    gamma = np.asarray(inputs["gamma"], np.float64)
    beta = np.asarray(inputs["beta"], np.float64)
    key = (N, QW, mask_dt_name, mode, not np.any(bia_b),
           bool((gamma == 1).all()), not beta.any())
    nc = _BUILD_CACHE.get(key)
    if nc is None:
        nc = _build(N, QW, mask_dt_name=mask_dt_name, mode=mode,
                    triv_bias=key[4], triv_gamma=key[5], triv_beta=key[6])
        _BUILD_CACHE[key] = nc
    old = nc.m
    nc.m = get_hw_module(nc.m)
    try:
        in_maps = []
        for core in range(N_CORES):
            q0 = core * QW
            in_maps.append({
                "xT": xT,
                "xq": np.ascontiguousarray(xT[:, q0:q0 + QW]),
                "wq": wq_h, "wk": wk_h, "wv": wv_h,
                "gamma_b": gam_b, "beta_b": bet_b, "bias_b": bia_b,
                "mall": malls[core],
            })
        res = run_bass_kernel_spmd(nc, in_maps, core_ids=list(range(N_CORES)),
                                   trace=trace, tmpdir=tmpdir)
    finally:
        nc.m = old
    out = np.concatenate([res.results[i]["out"] for i in range(N_CORES)], axis=0)
    return out.astype(np.float32), res


def kernel(**inputs) -> np.ndarray:
    out, _ = _run(inputs)
    return out


def _build2(N, mask_dt_name="bfloat16", mode="bf16"):
    """Hybrid sharding: core c owns head (c>>1) and row-half (c&1).

    Projections are per-head only (1/4 the replicated work of _build); the
    per-head partial outputs are summed across the 4 cores of each half via
    ReduceScatter (groups [[0,2,4,6],[1,3,5,7]]), one RS per 512-row
    superblock so all but the last overlap attention compute.  Core c's
    RS shard sb covers global rows (c&1)*2048 + sb*512 + (c>>1)*128.
    """
    import concourse.bacc as bacc
    import concourse.tile as tile
    import concourse.bass as bass
    from concourse import mybir

    f32 = mybir.dt.float32
    mask_dt = getattr(mybir.dt, mask_dt_name)
    cdt = {"f32r": mybir.dt.float32r, "bf16": mybir.dt.bfloat16,
           "f32": f32}[mode]
    Exp = mybir.ActivationFunctionType.Exp
    Copy = mybir.ActivationFunctionType.Copy
    Sqrt = mybir.ActivationFunctionType.Sqrt
    AX = mybir.AxisListType.X
    MUL = mybir.AluOpType.mult
    KV = N // P                 # kv chunks of 128
    HW_ = N // 2                # half-window width (2048)
    SBW = 512                   # superblock width
    NSB = HW_ // SBW            # superblocks (4)
    NB = N // 512
    D1 = D + 2

    nc = bacc.Bacc("TRN2", target_bir_lowering=False, debug=False,
                   num_devices=N_CORES)

    xT_d = nc.dram_tensor("xT", [D, N], cdt, kind="ExternalInput").ap()
    xq_d = nc.dram_tensor("xq", [D, HW_], cdt, kind="ExternalInput").ap()
    wq_d = nc.dram_tensor("wq", [D, D], cdt, kind="ExternalInput").ap()
    wk_d = nc.dram_tensor("wk", [D, D], cdt, kind="ExternalInput").ap()
    wv_d = nc.dram_tensor("wv", [D, D], cdt, kind="ExternalInput").ap()
    gam_d = nc.dram_tensor("gamma_b", [P, D], f32, kind="ExternalInput").ap()
    bet_d = nc.dram_tensor("beta_b", [P, D], f32, kind="ExternalInput").ap()
    bia_d = nc.dram_tensor("bias_b", [P, D], f32, kind="ExternalInput").ap()
    mal_d = nc.dram_tensor("mall", [P, NSB * KV * SBW], mask_dt,
                           kind="ExternalInput").ap()
    out_d = nc.dram_tensor("out", [NSB * P, D], f32, kind="ExternalOutput").ap()

    groups = [[0, 2, 4, 6], [1, 3, 5, 7]]

    with tile.TileContext(nc) as tc:
        with (
            tc.tile_pool(name="const", bufs=1) as cp,
            tc.tile_pool(name="maskp", bufs=2) as mp,
            tc.tile_pool(name="work", bufs=4) as wp,
            tc.tile_pool(name="accs", bufs=1) as ac,
            tc.tile_pool(name="ln", bufs=2) as lp,
            tc.tile_pool(name="psA", bufs=3, space="PSUM") as psA,
            tc.tile_pool(name="psO", bufs=1, space="PSUM") as psO,
            tc.tile_pool(name="dram", bufs=1, space="DRAM") as dp,
        ):
            wu = cp.tile([P, 640], mybir.dt.bfloat16, tag="wu")
            nc.vector.memset(wu[:], 0.125)
            wups = psA.tile([P, 512], f32, tag="ps", name="wups")
            for _ in range(16):
                nc.tensor.matmul(wups[:], lhsT=wu[:, :P], rhs=wu[:, P:P + 512],
                                 start=True, stop=True)

            xq = cp.tile([P, 2 * HW_], cdt, tag="xq")
            nc.sync.dma_start(out=xq[:].rearrange("p (i q) -> p i q", q=HW_),
                              in_=xq_d[:].rearrange("(i p) q -> p i q", p=P))
            wq = cp.tile([P, 2 * D], cdt, tag="wq")
            wk = cp.tile([P, 2 * D], cdt, tag="wk")
            wv = cp.tile([P, 2 * D], cdt, tag="wv")
            for wsb, wd in ((wq, wq_d), (wk, wk_d), (wv, wv_d)):
                nc.sync.dma_start(
                    out=wsb[:].rearrange("p (i d) -> p i d", i=2),
                    in_=wd[:].rearrange("(i p) d -> p i d", p=P))
            xT = cp.tile([P, 2 * N], cdt, tag="xT")
            nc.sync.dma_start(out=xT[:].rearrange("p (i n) -> p i n", n=N),
                              in_=xT_d[:].rearrange("(i p) n -> p i n", p=P))
            gam = cp.tile([P, D], f32, tag="gam")
            bet = cp.tile([P, D], f32, tag="bet")
            bia = cp.tile([P, D], f32, tag="bia")
            nc.sync.dma_start(out=gam[:], in_=gam_d[:])
            nc.sync.dma_start(out=bet[:], in_=bet_d[:])
            nc.sync.dma_start(out=bia[:], in_=bia_d[:])
            epsc = cp.tile([P, 1], f32, tag="epsc")
            nc.gpsimd.memset(epsc[:], EPS)
            onescol = cp.tile([P, 2 * KV], f32, tag="onescol")
            nc.gpsimd.memset(onescol[:], 1.0)

            # projections (single head)
            qT = cp.tile([P, 2 * HW_], cdt, tag="qT")
            for j in range(2):
                for qb in range(HW_ // 512):
                    ps = psA.tile([P, 512], f32, tag="ps")
                    for i in range(2):
                        nc.tensor.matmul(
                            ps[:],
                            lhsT=wq[:, i * D + j * P:i * D + j * P + P],
                            rhs=xq[:, i * HW_ + qb * 512:i * HW_ + (qb + 1) * 512],
                            start=(i == 0), stop=(i == 1))
                    nc.vector.tensor_copy(
                        qT[:, j * HW_ + qb * 512:j * HW_ + (qb + 1) * 512], ps[:])
            kT = cp.tile([P, 2 * N], cdt, tag="kT")
            for j in range(2):
                for b in range(NB):
                    ps = psA.tile([P, 512], f32, tag="ps")
                    for i in range(2):
                        nc.tensor.matmul(
                            ps[:],
                            lhsT=wk[:, i * D + j * P:i * D + j * P + P],
                            rhs=xT[:, i * N + b * 512:i * N + (b + 1) * 512],
                            start=(i == 0), stop=(i == 1))
                    if b % 2 == 0:
                        nc.scalar.copy(
                            kT[:, j * N + b * 512:j * N + (b + 1) * 512], ps[:])
                    else:
                        nc.vector.tensor_copy(
                            kT[:, j * N + b * 512:j * N + (b + 1) * 512], ps[:])
            vE = cp.tile([P, KV * D1], cdt, tag="vE")
            for c in range(KV):
                ps = psA.tile([P, 512], f32, tag="ps")
                for i in range(2):
                    nc.tensor.matmul(
                        ps[:, :D],
                        lhsT=xT[:, i * N + c * P:i * N + c * P + P],
                        rhs=wv[:, i * D:(i + 1) * D],
                        start=(i == 0), stop=(i == 1))
                nc.vector.tensor_copy(vE[:, c * D1:c * D1 + D], ps[:, :D])
            nc.vector.tensor_copy(
                vE[:].rearrange("p (c e) -> p c e", e=D1)[:, :, D:D + 2],
                onescol[:].rearrange("p (c e) -> p c e", e=2))

            # RS bounce buffers (one collective at the end)
            bounce_in = dp.tile([NSB * 4 * P, D], f32, name="bin")
            bounce_out = dp.tile([NSB * P, D], f32, name="bout")

            for sb in range(NSB):
                Msb = mp.tile([P, KV * SBW], mask_dt, tag="Msb",
                              name=f"Msb{sb}")
                for c in range(KV):
                    nc.sync.dma_start(
                        out=Msb[:, c * SBW:(c + 1) * SBW],
                        in_=mal_d[:, (sb * KV + c) * SBW:(sb * KV + c + 1) * SBW])
                oPS = [psO.tile([P, D1], f32, tag=f"oPS{s}", name=f"oPS{sb}_{s}")
                       for s in range(4)]

                def o_mms(c, et):
                    for s in range(4):
                        nc.tensor.matmul(oPS[s][:],
                                         lhsT=et[:, s * P:(s + 1) * P],
                                         rhs=vE[:, c * D1:(c + 1) * D1],
                                         start=(c == 0), stop=(c == KV - 1))

                prev = None
                for c in range(KV):
                    sc = psA.tile([P, 512], f32, tag="ps", name=f"sc{sb}_{c}")
                    for j in range(2):
                        nc.tensor.matmul(
                            sc[:],
                            lhsT=kT[:, j * N + c * P:j * N + c * P + P],
                            rhs=qT[:, j * HW_ + sb * SBW:j * HW_ + (sb + 1) * SBW],
                            start=(j == 0), stop=(j == 1))
                    et = wp.tile([P, SBW], cdt, tag="et", name=f"et{sb}_{c}")
                    nc.scalar.activation(et[:], sc[:], Exp, scale=1.0 / 16.0)
                    nc.vector.tensor_mul(et[:], et[:],
                                         Msb[:, c * SBW:(c + 1) * SBW])
                    if prev is not None:
                        o_mms(*prev)
                    prev = (c, et)
                o_mms(*prev)

                for s in range(4):
                    rec = lp.tile([P, 1], f32, tag="rec", name=f"rec{sb}_{s}")
                    nc.vector.reciprocal(rec[:], oPS[s][:, D:D + 1])
                    par = lp.tile([P, D], f32, tag="par", name=f"par{sb}_{s}")
                    nc.vector.tensor_scalar_mul(par[:], oPS[s][:, 0:D], rec[:])
                    nc.sync.dma_start(
                        out=bounce_in[(sb * 4 + s) * P:(sb * 4 + s + 1) * P, :],
                        in_=par[:])
            nc.gpsimd.collective_compute(
                "ReduceScatter", mybir.AluOpType.add,
                replica_groups=groups,
                ins=[bounce_in[:].opt()],
                outs=[bounce_out[:].opt()],
            )

            # post-RS: bias + LayerNorm + store, per superblock shard
            inv_d = 1.0 / D
            for sb in range(NSB):
                t = ac.tile([P, D], f32, tag="acc", name=f"post{sb}")
                nc.sync.dma_start(out=t[:],
                                  in_=bounce_out[sb * P:(sb + 1) * P, :])
                nc.vector.tensor_add(t[:], t[:], bia[:])
                musum = lp.tile([P, 1], f32, tag="musum", name=f"mus{sb}")
                nc.vector.reduce_sum(musum[:], t[:], axis=AX)
                mu = lp.tile([P, 1], f32, tag="mu", name=f"mu{sb}")
                nc.scalar.activation(mu[:], musum[:], Copy, scale=inv_d)
                xc = lp.tile([P, D], f32, tag="xc", name=f"xc{sb}")
                nc.vector.tensor_scalar_sub(xc[:], t[:], mu[:])
                sq = lp.tile([P, D], f32, tag="sq", name=f"sq{sb}")
                nc.vector.tensor_mul(sq[:], xc[:], xc[:])
                vs = lp.tile([P, 1], f32, tag="vs", name=f"vs{sb}")
                nc.vector.reduce_sum(vs[:], sq[:], axis=AX)
                sd = lp.tile([P, 1], f32, tag="sd", name=f"sd{sb}")
                nc.scalar.activation(sd[:], vs[:], Sqrt, bias=epsc[:],
                                     scale=inv_d)
                rs = lp.tile([P, 1], f32, tag="rs", name=f"rs{sb}")
                nc.vector.reciprocal(rs[:], sd[:])
                og = lp.tile([P, D], f32, tag="og", name=f"og{sb}")
                nc.vector.scalar_tensor_tensor(og[:], in0=xc[:], scalar=rs[:],
                                               in1=gam[:], op0=MUL, op1=MUL)
                oo = lp.tile([P, D], f32, tag="oo", name=f"oo{sb}")
                nc.vector.tensor_add(oo[:], og[:], bet[:])
                nc.sync.dma_start(out=out_d[sb * P:(sb + 1) * P, :], in_=oo[:])

    nc.compile()
    return nc


def _run2(inputs, trace=False, mask_dt_name="bfloat16", mode="bf16",
          tmpdir=None):
    from concourse.bass_utils import run_bass_kernel_spmd
    from concourse.bass_interp import get_hw_module
    import ml_dtypes

    N = int(np.asarray(inputs["x"]).shape[0])
    QW = N // N_CORES
    (xT, wq_h, wk_h, wv_h, gam_b, bet_b, bia_b, _malls) =         _prep_host(inputs, N, QW)
    ei = np.asarray(inputs["edge_index"]).astype(np.int64)
    adj = np.zeros((N, N), dtype=np.uint8)
    adj[ei[0], ei[1]] = 1
    adj[ei[1], ei[0]] = 1
    adj[np.arange(N), np.arange(N)] = 1

    if mode == "bf16":
        hdt = ml_dtypes.bfloat16
        xT = xT.astype(hdt)
        wq_h, wk_h, wv_h = (a.astype(hdt) for a in (wq_h, wk_h, wv_h))

    KV = N // P
    HW_ = N // 2
    SBW = 512
    NSB = HW_ // SBW
    nc = _build2(N, mask_dt_name=mask_dt_name, mode=mode)
    old = nc.m
    nc.m = get_hw_module(nc.m)
    try:
        in_maps = []
        for core in range(N_CORES):
            h, r = core >> 1, core & 1
            q0 = r * HW_
            stripe = adj[:, q0:q0 + HW_]
            m = np.zeros((P, NSB * KV * SBW), np.uint8)
            for sb in range(NSB):
                blk = stripe[:, sb * SBW:(sb + 1) * SBW]
                m[:, sb * KV * SBW:(sb + 1) * KV * SBW] = (
                    blk.reshape(KV, P, SBW).transpose(1, 0, 2)
                    .reshape(P, KV * SBW))
            in_maps.append({
                "xT": xT,
                "xq": np.ascontiguousarray(xT[:, q0:q0 + HW_]),
                "wq": wq_h[h], "wk": wk_h[h], "wv": wv_h[h],
                "gamma_b": gam_b, "beta_b": bet_b, "bias_b": bia_b,
                "mall": np.ascontiguousarray(m.astype(ml_dtypes.bfloat16)),
            })
        res = run_bass_kernel_spmd(nc, in_maps, core_ids=list(range(N_CORES)),
                                   trace=trace, tmpdir=tmpdir)
    finally:
        nc.m = old
    out = np.zeros((N, D), np.float32)
    for core in range(N_CORES):
        h, r = core >> 1, core & 1
        g0 = r * HW_ + h * (NSB * P)
        out[g0:g0 + NSB * P] = res.results[core]["out"]
    return out.astype(np.float32), res


# revision 35
# speedup vs baseline: 1.0065x; 1.0065x over previous
"""Trainium2 Bass kernel for nn_AdaptiveGraphConvLayer (graph multi-head attention).

Reference computation:
    mask = dense additive edge mask from edge_index (symmetric + self loops)
    per head h: q,k,v projections of x; scores = q @ k.T / 16 + mask; softmax
    o_h = attn @ v_h; head_out_h = o_h @ Wo_h.T + bo_h
    out = concat_h(head_out) @ Wp.T + bp;  LayerNorm(out) * gamma + beta
    (N=4096 nodes, D=256, H=4 heads, E=131072 edges; ~80 GFLOP)

Measured: ~227 us HW exec on 8 NeuronCores, rel err 3.2e-3 (bf16 matmuls,
fp32 accumulate/softmax/LayerNorm).

Device strategy (kernel(): node-parallel, zero collectives):
  - Core c owns query rows [c*512, (c+1)*512) for ALL 4 heads; k/v
    projections are recomputed per core.  On this setup a measured
    collective costs ~45-60 us (floor-dominated), more than the ~55 us of
    replicated projection matmuls it could remove, so the comm-free layout
    wins (a head-parallel + ReduceScatter variant, _build2/_run2, measured
    equal at best).
  - Algebraic fold: out = sum_h attn_h @ v'_h + bias_tot with
        v'_h = x @ (Wv_h^T (Wp_h Wo_h)^T)   (host-precomputed weight)
    which eliminates the per-head out-proj and final projection entirely.
  - scoresT blocks [kv=128, q=512] = kT-slices^T @ qT; exp on ACT
    (scale=1/16, no max-subtract needed: |scores| < ~1 and every row has a
    self loop); mask applied multiplicatively on DVE; softmax denominator
    via ones-columns appended to v' (o_ext[:, D] = row sum), normalized
    with a per-partition reciprocal.  o-matmuls run one kv-chunk behind the
    exp/mask pipeline so the PE never stalls.
  - Edge mask: host reshards edge_index into per-core dense {0,1} bf16
    stripes in SBUF layout (indirect-DMA scatter on real HW honors only one
    offset per partition per instruction, so an on-device build would cost
    ~260 serial SWDGE instructions ~ 300 us; host resharding keeps all
    FLOPs and all on-chip traffic on device).
  - bf16 everywhere on the PE (fast weight load; fp32r needs pre-rounded
    operands and loads weights 2x slower), fp32 PSUM accumulate, fp32
    softmax/normalize/LayerNorm.  psum->sbuf casts split across ACT/DVE.
  - Prologue: 40 dummy warmup matmuls keep the PE HAM clock-gate at 8/8
    through the input-DMA window; inputs land via few merged strided DMAs
    (sync-queue issue costs ~0.65 us per DMA instruction).
  - Tail: fused Square+accum_out variance, Sqrt table preloaded, affine
    LN ops elided when gamma/beta/bias are trivial for the given inputs.
"""

import numpy as np

N_FULL = 4096
D = 256
H = 4
N_CORES = 8
EPS = 1e-5
P = 128  # partitions


def _build(N, QW, mask_dt_name="bfloat16", mode="f32r",
           triv_bias=False, triv_gamma=False, triv_beta=False):
    """Build + compile the SPMD Bass graph (identical on all cores)."""
    import concourse.bacc as bacc
    import concourse.tile as tile
    import concourse.bass as bass
    from concourse import mybir

    f32 = mybir.dt.float32
    i32 = mybir.dt.int32
    mask_dt = getattr(mybir.dt, mask_dt_name)
    cdt = {"f32r": mybir.dt.float32r, "bf16": mybir.dt.bfloat16,
           "f32": f32}[mode]
    Exp = mybir.ActivationFunctionType.Exp
    Copy = mybir.ActivationFunctionType.Copy
    Sqrt = mybir.ActivationFunctionType.Sqrt
    AX = mybir.AxisListType.X
    MUL = mybir.AluOpType.mult
    KV = N // P            # kv chunks of 128
    QS = QW // P           # q slices of 128 within this core's window
    NB = N // 512          # 512-wide node blocks (kT projection)
    D1 = D + 2             # v' + ones columns (padded even for fp32r)

    def mc(ap):
        return ap

    nc = bacc.Bacc("TRN2", target_bir_lowering=False, debug=False,
                   num_devices=N_CORES)

    xT_d = nc.dram_tensor("xT", [D, N], cdt, kind="ExternalInput").ap()
    xq_d = nc.dram_tensor("xq", [D, QW], cdt, kind="ExternalInput").ap()
    wq_d = nc.dram_tensor("wq", [H, D, D], cdt, kind="ExternalInput").ap()
    wk_d = nc.dram_tensor("wk", [H, D, D], cdt, kind="ExternalInput").ap()
    wv_d = nc.dram_tensor("wv", [H, D, D], cdt, kind="ExternalInput").ap()
    gam_d = nc.dram_tensor("gamma_b", [P, D], f32, kind="ExternalInput").ap()
    bet_d = nc.dram_tensor("beta_b", [P, D], f32, kind="ExternalInput").ap()
    bia_d = nc.dram_tensor("bias_b", [P, D], f32, kind="ExternalInput").ap()
    mal_d = nc.dram_tensor("mall", [P, (N // P) * QW], mask_dt,
                           kind="ExternalInput").ap()
    out_d = nc.dram_tensor("out", [QW, D], f32, kind="ExternalOutput").ap()

    with tile.TileContext(nc) as tc:
        with (
            tc.tile_pool(name="const", bufs=1) as cp,
            tc.tile_pool(name="khead", bufs=2) as kp,
            tc.tile_pool(name="vhead", bufs=2) as vp,
            tc.tile_pool(name="maskp", bufs=1) as mp,
            tc.tile_pool(name="qhead", bufs=2) as qp,
            tc.tile_pool(name="work", bufs=4) as wp,
            tc.tile_pool(name="accs", bufs=1) as ac,
            tc.tile_pool(name="ln", bufs=2) as lp,
            tc.tile_pool(name="psA", bufs=4, space="PSUM") as psA,
            tc.tile_pool(name="psO", bufs=1, space="PSUM") as psO,
            tc.tile_pool(name="dram", bufs=1, space="DRAM") as dp,
        ):
            # ---------- PE warmup: dummy matmuls on uninitialized SBUF so
            # the HAM clock-gate reaches K=8/8 while input DMAs stream in.
            wu = cp.tile([P, 640], mybir.dt.bfloat16, tag="wu")
            nc.vector.memset(wu[:], 0.125)
            wups = psA.tile([P, 512], f32, tag="ps", name="wups")
            for _ in range(40):
                nc.tensor.matmul(wups[:], lhsT=wu[:, :P], rhs=wu[:, P:P + 512],
                                 start=True, stop=True)

            # ---------- load inputs into SBUF ----------
            # DMA queue is FIFO: land the q-projection inputs first so the
            # first real matmuls start as early as possible.
            xq = cp.tile([P, 2 * QW], cdt, tag="xq")
            nc.sync.dma_start(out=xq[:].rearrange("p (i q) -> p i q", q=QW),
                              in_=xq_d[:].rearrange("(i p) q -> p i q", p=P))
            wq = cp.tile([P, H * 2 * D], cdt, tag="wq")
            wk = cp.tile([P, H * 2 * D], cdt, tag="wk")
            wv = cp.tile([P, H * 2 * D], cdt, tag="wv")
            for wsb, wd in ((wq, wq_d), (wk, wk_d), (wv, wv_d)):
                nc.sync.dma_start(
                    out=wsb[:].rearrange("p (h i d) -> p h i d", h=H, i=2),
                    in_=wd[:].rearrange("h (i p) d -> p h i d", p=P))
            xT = cp.tile([P, 2 * N], cdt, tag="xT")
            NQ = N // 4
            for q4 in range(4):
                nc.sync.dma_start(
                    out=xT[:].rearrange("p (i n) -> p i n", n=N)
                        [:, :, q4 * NQ:(q4 + 1) * NQ],
                    in_=xT_d[:].rearrange("(i p) n -> p i n", p=P)
                        [:, :, q4 * NQ:(q4 + 1) * NQ])
            gam = cp.tile([P, D], f32, tag="gam")
            bet = cp.tile([P, D], f32, tag="bet")
            bia = cp.tile([P, D], f32, tag="bia")
            nc.sync.dma_start(out=gam[:], in_=gam_d[:])
            nc.sync.dma_start(out=bet[:], in_=bet_d[:])
            nc.sync.dma_start(out=bia[:], in_=bia_d[:])
            epsc = cp.tile([P, 1], f32, tag="epsc")
            nc.gpsimd.memset(epsc[:], EPS)
            eps2 = cp.tile([P, 1], f32, tag="eps2")
            nc.gpsimd.memset(eps2[:], float(D) * float(D) * EPS)
            onescol = cp.tile([P, 2 * KV], f32, tag="onescol")
            nc.gpsimd.memset(onescol[:], 1.0)
            sqwarm = cp.tile([P, 1], f32, tag="sqwarm")
            nc.scalar.activation(sqwarm[:], epsc[:], Sqrt, bias=epsc[:])

            # ---------- edge-mask stripe (host-sharded input) to SBUF ----
            # quarters: issued after inputs on the same queue; attention
            # chunk c waits only for its quarter
            Mall = mp.tile([P, KV * QW], mask_dt, tag="mask")
            MQ = KV // 4
            for q4 in range(4):
                nc.sync.dma_start(
                    out=Mall[:, q4 * MQ * QW:(q4 + 1) * MQ * QW],
                    in_=mal_d[:, q4 * MQ * QW:(q4 + 1) * MQ * QW])

            # ---------- per-head compute ----------
            acc = [ac.tile([P, D], f32, tag=f"acc{s}", name=f"acc{s}")
                   for s in range(QS)]

            def make_proj(h):
                """Allocate head-h tiles; return (tiles, emit-thunks).

                Each thunk emits one PSUM matmul pair + its psum->sbuf copy;
                thunks are interleaved into the previous head's attention so
                the copies spread over a window where DVE/ACT have slack."""
                qT = qp.tile([P, 2 * QW], cdt, tag="qT", name=f"qT{h}")
                kT = kp.tile([P, 2 * N], cdt, tag="kT", name=f"kT{h}")
                vE = vp.tile([P, KV * D1], cdt, tag="vE", name=f"vE{h}")
                ops = []
                eng = [0]

                def qT_pair(j):
                    ps = psA.tile([P, 512], f32, tag="ps", name=f"q{h}_{j}")
                    for i in range(2):
                        w = (h * 2 + i) * D + j * P
                        nc.tensor.matmul(ps[:, :QW], lhsT=wq[:, w:w + P],
                                         rhs=xq[:, i * QW:(i + 1) * QW],
                                         start=(i == 0), stop=(i == 1))
                    nc.vector.tensor_copy(qT[:, j * QW:(j + 1) * QW],
                                          ps[:, :QW])

                def kT_pair(b, j):
                    ps = psA.tile([P, 512], f32, tag="ps", name=f"k{h}_{b}_{j}")
                    for i in range(2):
                        w = (h * 2 + i) * D + j * P
                        nc.tensor.matmul(
                            ps[:], lhsT=wk[:, w:w + P],
                            rhs=xT[:, i * N + b * 512:i * N + (b + 1) * 512],
                            start=(i == 0), stop=(i == 1))
                    dst = kT[:, j * N + b * 512:j * N + (b + 1) * 512]
                    if eng[0] % 2 == 0:
                        nc.scalar.copy(dst, ps[:])
                    else:
                        nc.vector.tensor_copy(dst, ps[:])
                    eng[0] += 1

                def vE_pair(c):
                    ps = psA.tile([P, 512], f32, tag="ps", name=f"v{h}_{c}")
                    for i in range(2):
                        nc.tensor.matmul(
                            ps[:, :D],
                            lhsT=xT[:, i * N + c * P:i * N + c * P + P],
                            rhs=wv[:, (h * 2 + i) * D:(h * 2 + i + 1) * D],
                            start=(i == 0), stop=(i == 1))
                    # head 0 runs upfront with ACT otherwise idle (no exp yet):
                    # split its copies across both engines; later heads keep
                    # vE on DVE so ACT has headroom for the interleaved exp
                    if h == 0 and c % 2 == 0:
                        nc.scalar.copy(vE[:, c * D1:c * D1 + D], ps[:, :D])
                    else:
                        nc.vector.tensor_copy(vE[:, c * D1:c * D1 + D],
                                              ps[:, :D])

                def ones_cols():
                    nc.vector.tensor_copy(
                        vE[:].rearrange("p (c e) -> p c e", e=D1)[:, :, D:D + 2],
                        onescol[:].rearrange("p (c e) -> p c e", e=2))

                for j in range(2):
                    ops.append(lambda j=j: qT_pair(j))
                for b in range(NB):
                    for j in range(2):
                        ops.append(lambda b=b, j=j: kT_pair(b, j))
                for c in range(KV):
                    ops.append(lambda c=c: vE_pair(c))
                ops.append(ones_cols)
                return (qT, kT, vE), ops

            cur, ops0 = make_proj(0)
            for op in ops0:
                op()

            for h in range(H):
                qT, kT, vE = cur
                if h + 1 < H:
                    nxt, pend = make_proj(h + 1)
                else:
                    nxt, pend = None, []
                per_chunk = -(-len(pend) // (KV - 4)) if pend else 0

                oPS = [psO.tile([P, D1], f32, tag=f"oPS{s}", name=f"oPS{s}")
                       for s in range(QS)]

                def o_mms(c, et):
                    for s in range(QS):
                        nc.tensor.matmul(oPS[s][:],
                                         lhsT=et[:, s * P:(s + 1) * P],
                                         rhs=vE[:, c * D1:(c + 1) * D1],
                                         start=(c == 0), stop=(c == KV - 1))

                prev = None
                for c in range(KV):
                    sc = psA.tile([P, 512], f32, tag="ps")
                    for j in range(2):
                        nc.tensor.matmul(sc[:, :QW],
                                         lhsT=kT[:, j * N + c * P:j * N + c * P + P],
                                         rhs=qT[:, j * QW:(j + 1) * QW],
                                         start=(j == 0), stop=(j == 1))
                    et = wp.tile([P, QW], cdt, tag="et")
                    nc.scalar.activation(et[:], sc[:, :QW], Exp, scale=1.0 / 16.0)
                    nc.vector.tensor_mul(et[:], et[:], Mall[:, c * QW:(c + 1) * QW])
                    if prev is not None:
                        o_mms(*prev)
                    prev = (c, et)
                    if c >= 4:
                        for _ in range(per_chunk):
                            if pend:
                                pend.pop(0)()
                o_mms(*prev)
                while pend:
                    pend.pop(0)()

                for s in range(QS):
                    rec = lp.tile([P, 1], f32, tag="rec")
                    nc.vector.reciprocal(rec[:], oPS[s][:, D:D + 1])
                    if h == 0:
                        nc.vector.tensor_scalar_mul(acc[s][:], oPS[s][:, 0:D], rec[:])
                    else:
                        tmp = lp.tile([P, D], f32, tag="tmp")
                        nc.vector.tensor_scalar_mul(tmp[:], oPS[s][:, 0:D], rec[:])
                        nc.vector.tensor_add(acc[s][:], acc[s][:], tmp[:])
                cur = nxt

            # ---------- bias + LayerNorm + store ----------
            inv_d = 1.0 / D
            Square = mybir.ActivationFunctionType.Square
            for s in range(QS):
                t = acc[s]
                if not triv_bias:
                    nc.vector.tensor_add(t[:], t[:], bia[:])
                musum = lp.tile([P, 1], f32, tag="musum")
                nc.vector.reduce_sum(musum[:], t[:], axis=AX)
                # LN is scale-invariant: center as D*t - sum(t), compensate in
                # the sqrt (scale 1/D, bias D^2*eps) -- one op fewer per slice
                xc = lp.tile([P, D], f32, tag="xc")
                nc.vector.tensor_scalar(out=xc[:], in0=t[:], scalar1=float(D),
                                        scalar2=musum[:],
                                        op0=MUL, op1=mybir.AluOpType.subtract)
                sq = lp.tile([P, D], f32, tag="sq")
                vs = lp.tile([P, 1], f32, tag="vs")
                nc.scalar.activation(sq[:], xc[:], Square, accum_out=vs[:])
                sd = lp.tile([P, 1], f32, tag="sd")
                nc.scalar.activation(sd[:], vs[:], Sqrt, bias=eps2[:], scale=inv_d)
                rs = lp.tile([P, 1], f32, tag="rs")
                nc.vector.reciprocal(rs[:], sd[:])
                og = lp.tile([P, D], f32, tag="og")
                if triv_gamma:
                    nc.vector.tensor_scalar_mul(og[:], xc[:], rs[:])
                else:
                    nc.vector.scalar_tensor_tensor(og[:], in0=xc[:],
                                                   scalar=rs[:], in1=gam[:],
                                                   op0=MUL, op1=MUL)
                if triv_beta:
                    nc.sync.dma_start(out=out_d[s * P:(s + 1) * P, :], in_=og[:])
                else:
                    oo = lp.tile([P, D], f32, tag="oo")
                    nc.vector.tensor_add(oo[:], og[:], bet[:])
                    nc.sync.dma_start(out=out_d[s * P:(s + 1) * P, :], in_=oo[:])

    nc.compile()
    return nc


def _prep_host(inputs, N, QW):
    """Host-side input resharding: transposes, folded weights, mask offsets."""
    x = np.ascontiguousarray(np.asarray(inputs["x"], dtype=np.float32))
    ei = np.asarray(inputs["edge_index"]).astype(np.int64)
    Wq = np.asarray(inputs["Wq"], dtype=np.float64)
    Wk = np.asarray(inputs["Wk"], dtype=np.float64)
    Wv = np.asarray(inputs["Wv"], dtype=np.float64)
    Wo = np.asarray(inputs["Wo"], dtype=np.float64)
    Wp = np.asarray(inputs["Wp"], dtype=np.float64)
    bq = np.asarray(inputs["bq"], dtype=np.float64)
    bk = np.asarray(inputs["bk"], dtype=np.float64)
    bv = np.asarray(inputs["bv"], dtype=np.float64)
    bo = np.asarray(inputs["bo"], dtype=np.float64)
    bp = np.asarray(inputs["bp"], dtype=np.float64)
    gamma = np.asarray(inputs["gamma"], dtype=np.float32)
    beta = np.asarray(inputs["beta"], dtype=np.float32)

    assert not bq.any() and not bk.any(), \
        "nonzero q/k biases not wired in the device graph"

    xT = np.ascontiguousarray(x.T)                       # [D, N]
    wq_h = np.ascontiguousarray(
        np.stack([Wq[h].T for h in range(H)]).astype(np.float32))
    wk_h = np.ascontiguousarray(
        np.stack([Wk[h].T for h in range(H)]).astype(np.float32))
    # folded v' weight and total bias
    wv_l, bias_tot = [], bp.copy()
    for h in range(H):
        Wp_h = Wp[:, h * D:(h + 1) * D]                  # [f, e']
        G = Wo[h].T @ Wp_h.T                             # [e, f]
        wv_l.append(Wv[h].T @ G)                         # [d, f]
        bias_tot = bias_tot + bo[h] @ Wp_h.T + bv[h] @ G
    wv_h = np.ascontiguousarray(np.stack(wv_l).astype(np.float32))

    gam_b = np.ascontiguousarray(np.broadcast_to(gamma, (P, D)).astype(np.float32))
    bet_b = np.ascontiguousarray(np.broadcast_to(beta, (P, D)).astype(np.float32))
    bia_b = np.ascontiguousarray(
        np.broadcast_to(bias_tot.astype(np.float32), (P, D)))

    # mask stripes per core, pre-arranged to the SBUF layout
    # mall[p, c*QW + q] = adjacency[c*P + p, q0 + q]  (kv-major, symmetric+diag)
    import ml_dtypes
    adj = np.zeros((N, N), dtype=np.uint8)
    r, c = ei[0], ei[1]
    adj[r, c] = 1
    adj[c, r] = 1
    adj[np.arange(N), np.arange(N)] = 1
    KV = N // P
    malls = []
    for core in range(N_CORES):
        q0 = core * QW
        stripe = adj[:, q0:q0 + QW]                      # [N(kv), QW]
        m = stripe.reshape(KV, P, QW).transpose(1, 0, 2).reshape(P, KV * QW)
        malls.append(np.ascontiguousarray(m.astype(ml_dtypes.bfloat16)))
    return xT, wq_h, wk_h, wv_h, gam_b, bet_b, bia_b, malls


def _run(inputs, trace=False, mask_dt_name="bfloat16", mode="f32r",
         tmpdir=None):
    from concourse.bass_utils import run_bass_kernel_spmd
    from concourse.bass_interp import get_hw_module

    N = int(np.asarray(inputs["x"]).shape[0])
    QW = N // N_CORES
    (xT, wq_h, wk_h, wv_h, gam_b, bet_b, bia_b, malls) = \
        _prep_host(inputs, N, QW)

    if mode == "bf16":
        import ml_dtypes
        hdt = ml_dtypes.bfloat16
        xT = xT.astype(hdt)
        wq_h, wk_h, wv_h = (a.astype(hdt) for a in (wq_h, wk_h, wv_h))
    elif mode == "f32r":
        # fp32r operands must be pre-rounded (RNE dropping 12 mantissa bits);
        # matches walrus fp32_to_fp32r.
        def _r(a):
            b = a.view(np.uint32).astype(np.uint64)
            rb = (b + 0x7FF + ((b >> 12) & 1)) & np.uint64(0xFFFFF000)
            return rb.astype(np.uint32).view(np.float32)
        xT = _r(xT)
        wq_h, wk_h, wv_h = _r(wq_h), _r(wk_h), _r(wv_h)
    gamma = np.asarray(inputs["gamma"], np.float64)
    beta = np.asarray(inputs["beta"], np.float64)
    key = (N, QW, mask_dt_name, mode, not np.any(bia_b),
           bool((gamma == 1).all()), not beta.any())
    nc = _BUILD_CACHE.get(key)
    if nc is None:
        nc = _build(N, QW, mask_dt_name=mask_dt_name, mode=mode,
                    triv_bias=key[4], triv_gamma=key[5], triv_beta=key[6])
        _BUILD_CACHE[key] = nc
    old = nc.m
    nc.m = get_hw_module(nc.m)
    try:
        in_maps = []
        for core in range(N_CORES):
            q0 = core * QW
            in_maps.append({
                "xT": xT,
                "xq": np.ascontiguousarray(xT[:, q0:q0 + QW]),
                "wq": wq_h, "wk": wk_h, "wv": wv_h,
                "gamma_b": gam_b, "beta_b": bet_b, "bias_b": bia_b,
                "mall": malls[core],
            })
        res = run_bass_kernel_spmd(nc, in_maps, core_ids=list(range(N_CORES)),
                                   trace=trace, tmpdir=tmpdir)
    finally:
        nc.m = old
    out = np.concatenate([res.results[i]["out"] for i in range(N_CORES)], axis=0)
    return out.astype(np.float32), res


def kernel(**inputs) -> np.ndarray:
    out, _ = _run(inputs)
    return out


def _build2(N, mask_dt_name="bfloat16", mode="bf16"):
    """Hybrid sharding: core c owns head (c>>1) and row-half (c&1).

    Projections are per-head only (1/4 the replicated work of _build); the
    per-head partial outputs are summed across the 4 cores of each half via
    ReduceScatter (groups [[0,2,4,6],[1,3,5,7]]), one RS per 512-row
    superblock so all but the last overlap attention compute.  Core c's
    RS shard sb covers global rows (c&1)*2048 + sb*512 + (c>>1)*128.
    """
    import concourse.bacc as bacc
    import concourse.tile as tile
    import concourse.bass as bass
    from concourse import mybir

    f32 = mybir.dt.float32
    mask_dt = getattr(mybir.dt, mask_dt_name)
    cdt = {"f32r": mybir.dt.float32r, "bf16": mybir.dt.bfloat16,
           "f32": f32}[mode]
    Exp = mybir.ActivationFunctionType.Exp
    Copy = mybir.ActivationFunctionType.Copy
    Sqrt = mybir.ActivationFunctionType.Sqrt
    AX = mybir.AxisListType.X
    MUL = mybir.AluOpType.mult
    KV = N // P                 # kv chunks of 128
    HW_ = N // 2                # half-window width (2048)
    SBW = 512                   # superblock width
    NSB = HW_ // SBW            # superblocks (4)
    NB = N // 512
    D1 = D + 2

    nc = bacc.Bacc("TRN2", target_bir_lowering=False, debug=False,
                   num_devices=N_CORES)

    xT_d = nc.dram_tensor("xT", [D, N], cdt, kind="ExternalInput").ap()
    xq_d = nc.dram_tensor("xq", [D, HW_], cdt, kind="ExternalInput").ap()
    wq_d = nc.dram_tensor("wq", [D, D], cdt, kind="ExternalInput").ap()
    wk_d = nc.dram_tensor("wk", [D, D], cdt, kind="ExternalInput").ap()
    wv_d = nc.dram_tensor("wv", [D, D], cdt, kind="ExternalInput").ap()
    gam_d = nc.dram_tensor("gamma_b", [P, D], f32, kind="ExternalInput").ap()
    bet_d = nc.dram_tensor("beta_b", [P, D], f32, kind="ExternalInput").ap()
    bia_d = nc.dram_tensor("bias_b", [P, D], f32, kind="ExternalInput").ap()
    mal_d = nc.dram_tensor("mall", [P, NSB * KV * SBW], mask_dt,
                           kind="ExternalInput").ap()
    out_d = nc.dram_tensor("out", [NSB * P, D], f32, kind="ExternalOutput").ap()

    groups = [[0, 2, 4, 6], [1, 3, 5, 7]]

    with tile.TileContext(nc) as tc:
        with (
            tc.tile_pool(name="const", bufs=1) as cp,
            tc.tile_pool(name="maskp", bufs=2) as mp,
            tc.tile_pool(name="work", bufs=4) as wp,
            tc.tile_pool(name="accs", bufs=1) as ac,
            tc.tile_pool(name="ln", bufs=2) as lp,
            tc.tile_pool(name="psA", bufs=3, space="PSUM") as psA,
            tc.tile_pool(name="psO", bufs=1, space="PSUM") as psO,
            tc.tile_pool(name="dram", bufs=1, space="DRAM") as dp,
        ):
            wu = cp.tile([P, 640], mybir.dt.bfloat16, tag="wu")
            nc.vector.memset(wu[:], 0.125)
            wups = psA.tile([P, 512], f32, tag="ps", name="wups")
            for _ in range(16):
                nc.tensor.matmul(wups[:], lhsT=wu[:, :P], rhs=wu[:, P:P + 512],
                                 start=True, stop=True)

            xq = cp.tile([P, 2 * HW_], cdt, tag="xq")
            nc.sync.dma_start(out=xq[:].rearrange("p (i q) -> p i q", q=HW_),
                              in_=xq_d[:].rearrange("(i p) q -> p i q", p=P))
            wq = cp.tile([P, 2 * D], cdt, tag="wq")
            wk = cp.tile([P, 2 * D], cdt, tag="wk")
            wv = cp.tile([P, 2 * D], cdt, tag="wv")
            for wsb, wd in ((wq, wq_d), (wk, wk_d), (wv, wv_d)):
                nc.sync.dma_start(
                    out=wsb[:].rearrange("p (i d) -> p i d", i=2),
                    in_=wd[:].rearrange("(i p) d -> p i d", p=P))
            xT = cp.tile([P, 2 * N], cdt, tag="xT")
            nc.sync.dma_start(out=xT[:].rearrange("p (i n) -> p i n", n=N),
                              in_=xT_d[:].rearrange("(i p) n -> p i n", p=P))
            gam = cp.tile([P, D], f32, tag="gam")
            bet = cp.tile([P, D], f32, tag="bet")
            bia = cp.tile([P, D], f32, tag="bia")
            nc.sync.dma_start(out=gam[:], in_=gam_d[:])
            nc.sync.dma_start(out=bet[:], in_=bet_d[:])
            nc.sync.dma_start(out=bia[:], in_=bia_d[:])
            epsc = cp.tile([P, 1], f32, tag="epsc")
            nc.gpsimd.memset(epsc[:], EPS)
            onescol = cp.tile([P, 2 * KV], f32, tag="onescol")
            nc.gpsimd.memset(onescol[:], 1.0)

            # projections (single head)
            qT = cp.tile([P, 2 * HW_], cdt, tag="qT")
            for j in range(2):
                for qb in range(HW_ // 512):
                    ps = psA.tile([P, 512], f32, tag="ps")
                    for i in range(2):
                        nc.tensor.matmul(
                            ps[:],
                            lhsT=wq[:, i * D + j * P:i * D + j * P + P],
                            rhs=xq[:, i * HW_ + qb * 512:i * HW_ + (qb + 1) * 512],
                            start=(i == 0), stop=(i == 1))
                    nc.vector.tensor_copy(
                        qT[:, j * HW_ + qb * 512:j * HW_ + (qb + 1) * 512], ps[:])
            kT = cp.tile([P, 2 * N], cdt, tag="kT")
            for j in range(2):
                for b in range(NB):
                    ps = psA.tile([P, 512], f32, tag="ps")
                    for i in range(2):
                        nc.tensor.matmul(
                            ps[:],
                            lhsT=wk[:, i * D + j * P:i * D + j * P + P],
                            rhs=xT[:, i * N + b * 512:i * N + (b + 1) * 512],
                            start=(i == 0), stop=(i == 1))
                    if b % 2 == 0:
                        nc.scalar.copy(
                            kT[:, j * N + b * 512:j * N + (b + 1) * 512], ps[:])
                    else:
                        nc.vector.tensor_copy(
                            kT[:, j * N + b * 512:j * N + (b + 1) * 512], ps[:])
            vE = cp.tile([P, KV * D1], cdt, tag="vE")
            for c in range(KV):
                ps = psA.tile([P, 512], f32, tag="ps")
                for i in range(2):
                    nc.tensor.matmul(
                        ps[:, :D],
                        lhsT=xT[:, i * N + c * P:i * N + c * P + P],
                        rhs=wv[:, i * D:(i + 1) * D],
                        start=(i == 0), stop=(i == 1))
                nc.vector.tensor_copy(vE[:, c * D1:c * D1 + D], ps[:, :D])
            nc.vector.tensor_copy(
                vE[:].rearrange("p (c e) -> p c e", e=D1)[:, :, D:D + 2],
                onescol[:].rearrange("p (c e) -> p c e", e=2))

            # RS bounce buffers (one collective at the end)
            bounce_in = dp.tile([NSB * 4 * P, D], f32, name="bin")
            bounce_out = dp.tile([NSB * P, D], f32, name="bout")

            for sb in range(NSB):
                Msb = mp.tile([P, KV * SBW], mask_dt, tag="Msb",
                              name=f"Msb{sb}")
                for c in range(KV):
                    nc.sync.dma_start(
                        out=Msb[:, c * SBW:(c + 1) * SBW],
                        in_=mal_d[:, (sb * KV + c) * SBW:(sb * KV + c + 1) * SBW])
                oPS = [psO.tile([P, D1], f32, tag=f"oPS{s}", name=f"oPS{sb}_{s}")
                       for s in range(4)]

                def o_mms(c, et):
                    for s in range(4):
                        nc.tensor.matmul(oPS[s][:],
                                         lhsT=et[:, s * P:(s + 1) * P],
                                         rhs=vE[:, c * D1:(c + 1) * D1],
                                         start=(c == 0), stop=(c == KV - 1))

                prev = None
                for c in range(KV):
                    sc = psA.tile([P, 512], f32, tag="ps", name=f"sc{sb}_{c}")
                    for j in range(2):
                        nc.tensor.matmul(
                            sc[:],
                            lhsT=kT[:, j * N + c * P:j * N + c * P + P],
                            rhs=qT[:, j * HW_ + sb * SBW:j * HW_ + (sb + 1) * SBW],
                            start=(j == 0), stop=(j == 1))
                    et = wp.tile([P, SBW], cdt, tag="et", name=f"et{sb}_{c}")
                    nc.scalar.activation(et[:], sc[:], Exp, scale=1.0 / 16.0)
                    nc.vector.tensor_mul(et[:], et[:],
                                         Msb[:, c * SBW:(c + 1) * SBW])
                    if prev is not None:
                        o_mms(*prev)
                    prev = (c, et)
                o_mms(*prev)

                for s in range(4):
                    rec = lp.tile([P, 1], f32, tag="rec", name=f"rec{sb}_{s}")
                    nc.vector.reciprocal(rec[:], oPS[s][:, D:D + 1])
                    par = lp.tile([P, D], f32, tag="par", name=f"par{sb}_{s}")
                    nc.vector.tensor_scalar_mul(par[:], oPS[s][:, 0:D], rec[:])
                    nc.sync.dma_start(
                        out=bounce_in[(sb * 4 + s) * P:(sb * 4 + s + 1) * P, :],
                        in_=par[:])
            nc.gpsimd.collective_compute(
                "ReduceScatter", mybir.AluOpType.add,
                replica_groups=groups,
                ins=[bounce_in[:].opt()],
                outs=[bounce_out[:].opt()],
            )

            # post-RS: bias + LayerNorm + store, per superblock shard
            inv_d = 1.0 / D
            for sb in range(NSB):
                t = ac.tile([P, D], f32, tag="acc", name=f"post{sb}")
                nc.sync.dma_start(out=t[:],
                                  in_=bounce_out[sb * P:(sb + 1) * P, :])
                nc.vector.tensor_add(t[:], t[:], bia[:])
                musum = lp.tile([P, 1], f32, tag="musum", name=f"mus{sb}")
                nc.vector.reduce_sum(musum[:], t[:], axis=AX)
                mu = lp.tile([P, 1], f32, tag="mu", name=f"mu{sb}")
                nc.scalar.activation(mu[:], musum[:], Copy, scale=inv_d)
                xc = lp.tile([P, D], f32, tag="xc", name=f"xc{sb}")
                nc.vector.tensor_scalar_sub(xc[:], t[:], mu[:])
                sq = lp.tile([P, D], f32, tag="sq", name=f"sq{sb}")
                nc.vector.tensor_mul(sq[:], xc[:], xc[:])
                vs = lp.tile([P, 1], f32, tag="vs", name=f"vs{sb}")
                nc.vector.reduce_sum(vs[:], sq[:], axis=AX)
                sd = lp.tile([P, 1], f32, tag="sd", name=f"sd{sb}")
                nc.scalar.activation(sd[:], vs[:], Sqrt, bias=epsc[:],
                                     scale=inv_d)
                rs = lp.tile([P, 1], f32, tag="rs", name=f"rs{sb}")
                nc.vector.reciprocal(rs[:], sd[:])
                og = lp.tile([P, D], f32, tag="og", name=f"og{sb}")
                nc.vector.scalar_tensor_tensor(og[:], in0=xc[:], scalar=rs[:],
                                               in1=gam[:], op0=MUL, op1=MUL)
                oo = lp.tile([P, D], f32, tag="oo", name=f"oo{sb}")
                nc.vector.tensor_add(oo[:], og[:], bet[:])
                nc.sync.dma_start(out=out_d[sb * P:(sb + 1) * P, :], in_=oo[:])

    nc.compile()
    return nc


def _run2(inputs, trace=False, mask_dt_name="bfloat16", mode="bf16",
          tmpdir=None):
    from concourse.bass_utils import run_bass_kernel_spmd
    from concourse.bass_interp import get_hw_module
    import ml_dtypes

    N = int(np.asarray(inputs["x"]).shape[0])
    QW = N // N_CORES
    (xT, wq_h, wk_h, wv_h, gam_b, bet_b, bia_b, _malls) =         _prep_host(inputs, N, QW)
    ei = np.asarray(inputs["edge_index"]).astype(np.int64)
    adj = np.zeros((N, N), dtype=np.uint8)
    adj[ei[0], ei[1]] = 1
    adj[ei[1], ei[0]] = 1
    adj[np.arange(N), np.arange(N)] = 1

    if mode == "bf16":
        hdt = ml_dtypes.bfloat16
        xT = xT.astype(hdt)
        wq_h, wk_h, wv_h = (a.astype(hdt) for a in (wq_h, wk_h, wv_h))

    KV = N // P
    HW_ = N // 2
    SBW = 512
    NSB = HW_ // SBW
    nc = _build2(N, mask_dt_name=mask_dt_name, mode=mode)
    old = nc.m
    nc.m = get_hw_module(nc.m)
    try:
        in_maps = []
        for core in range(N_CORES):
            h, r = core >> 1, core & 1
            q0 = r * HW_
            stripe = adj[:, q0:q0 + HW_]
            m = np.zeros((P, NSB * KV * SBW), np.uint8)
            for sb in range(NSB):
                blk = stripe[:, sb * SBW:(sb + 1) * SBW]
                m[:, sb * KV * SBW:(sb + 1) * KV * SBW] = (
                    blk.reshape(KV, P, SBW).transpose(1, 0, 2)
                    .reshape(P, KV * SBW))
            in_maps.append({
                "xT": xT,
                "xq": np.ascontiguousarray(xT[:, q0:q0 + HW_]),
                "wq": wq_h[h], "wk": wk_h[h], "wv": wv_h[h],
                "gamma_b": gam_b, "beta_b": bet_b, "bias_b": bia_b,
                "mall": np.ascontiguousarray(m.astype(ml_dtypes.bfloat16)),
            })
        res = run_bass_kernel_spmd(nc, in_maps, core_ids=list(range(N_CORES)),
                                   trace=trace, tmpdir=tmpdir)
    finally:
        nc.m = old
    out = np.zeros((N, D), np.float32)
    for core in range(N_CORES):
        h, r = core >> 1, core & 1
        g0 = r * HW_ + h * (NSB * P)
        out[g0:g0 + NSB * P] = res.results[core]["out"]
    return out.astype(np.float32), res


# revision 36
# speedup vs baseline: 1.0098x; 1.0033x over previous
"""Trainium2 Bass kernel for nn_AdaptiveGraphConvLayer (graph multi-head attention).

Reference computation:
    mask = dense additive edge mask from edge_index (symmetric + self loops)
    per head h: q,k,v projections of x; scores = q @ k.T / 16 + mask; softmax
    o_h = attn @ v_h; head_out_h = o_h @ Wo_h.T + bo_h
    out = concat_h(head_out) @ Wp.T + bp;  LayerNorm(out) * gamma + beta
    (N=4096 nodes, D=256, H=4 heads, E=131072 edges; ~80 GFLOP)

Measured: ~210 us HW exec on 8 NeuronCores at full clock (chip P0 power
throttling, when active after sustained load, scales this ~1.2x), rel err
3.2e-3 (bf16 matmuls, fp32 accumulate/softmax/LayerNorm).

Device strategy (kernel(): node-parallel, zero collectives):
  - Core c owns query rows [c*512, (c+1)*512) for ALL 4 heads; k/v
    projections are recomputed per core.  On this setup a measured
    collective costs ~45-60 us (floor-dominated), more than the ~55 us of
    replicated projection matmuls it could remove, so the comm-free layout
    wins (a head-parallel + ReduceScatter variant, _build2/_run2, measured
    equal at best).
  - Algebraic fold: out = sum_h attn_h @ v'_h + bias_tot with
        v'_h = x @ (Wv_h^T (Wp_h Wo_h)^T)   (host-precomputed weight)
    which eliminates the per-head out-proj and final projection entirely.
  - scoresT blocks [kv=128, q=512] = kT-slices^T @ qT; exp on ACT
    (scale=1/16, no max-subtract needed: |scores| < ~1 and every row has a
    self loop); mask applied multiplicatively on DVE; softmax denominator
    via ones-columns appended to v' (o_ext[:, D] = row sum), normalized
    with a per-partition reciprocal.  o-matmuls run one kv-chunk behind the
    exp/mask pipeline so the PE never stalls.
  - Edge mask: host reshards edge_index into per-core dense {0,1} bf16
    stripes in SBUF layout (indirect-DMA scatter on real HW honors only one
    offset per partition per instruction, so an on-device build would cost
    ~260 serial SWDGE instructions ~ 300 us; host resharding keeps all
    FLOPs and all on-chip traffic on device).
  - bf16 everywhere on the PE (fast weight load; fp32r needs pre-rounded
    operands and loads weights 2x slower), fp32 PSUM accumulate, fp32
    softmax/normalize/LayerNorm.  psum->sbuf casts split across ACT/DVE.
  - Head h+1's projection matmul pairs (and their psum->sbuf copies) are
    emitted interleaved into head h's attention kv-loop, spreading the
    copy work over a window where DVE/ACT have slack -- this removed
    ~0.5-1.2 us copy-backpressure stalls per projection pair at head
    boundaries (-16 us total).
  - Prologue: 40 dummy warmup matmuls keep the PE HAM clock-gate at 8/8
    through the input-DMA window; inputs land via few merged strided DMAs
    (sync-queue issue costs ~0.65 us per DMA instruction).
  - Tail: fused Square+accum_out variance, Sqrt table preloaded, affine
    LN ops elided when gamma/beta/bias are trivial for the given inputs.
"""

import numpy as np

N_FULL = 4096
D = 256
H = 4
N_CORES = 8
EPS = 1e-5
P = 128  # partitions


def _build(N, QW, mask_dt_name="bfloat16", mode="f32r",
           triv_bias=False, triv_gamma=False, triv_beta=False):
    """Build + compile the SPMD Bass graph (identical on all cores)."""
    import concourse.bacc as bacc
    import concourse.tile as tile
    import concourse.bass as bass
    from concourse import mybir

    f32 = mybir.dt.float32
    i32 = mybir.dt.int32
    mask_dt = getattr(mybir.dt, mask_dt_name)
    cdt = {"f32r": mybir.dt.float32r, "bf16": mybir.dt.bfloat16,
           "f32": f32}[mode]
    Exp = mybir.ActivationFunctionType.Exp
    Copy = mybir.ActivationFunctionType.Copy
    Sqrt = mybir.ActivationFunctionType.Sqrt
    AX = mybir.AxisListType.X
    MUL = mybir.AluOpType.mult
    KV = N // P            # kv chunks of 128
    QS = QW // P           # q slices of 128 within this core's window
    NB = N // 512          # 512-wide node blocks (kT projection)
    D1 = D + 2             # v' + ones columns (padded even for fp32r)

    def mc(ap):
        return ap

    nc = bacc.Bacc("TRN2", target_bir_lowering=False, debug=False,
                   num_devices=N_CORES)

    xT_d = nc.dram_tensor("xT", [D, N], cdt, kind="ExternalInput").ap()
    xq_d = nc.dram_tensor("xq", [D, QW], cdt, kind="ExternalInput").ap()
    wq_d = nc.dram_tensor("wq", [H, D, D], cdt, kind="ExternalInput").ap()
    wk_d = nc.dram_tensor("wk", [H, D, D], cdt, kind="ExternalInput").ap()
    wv_d = nc.dram_tensor("wv", [H, D, D], cdt, kind="ExternalInput").ap()
    gam_d = nc.dram_tensor("gamma_b", [P, D], f32, kind="ExternalInput").ap()
    bet_d = nc.dram_tensor("beta_b", [P, D], f32, kind="ExternalInput").ap()
    bia_d = nc.dram_tensor("bias_b", [P, D], f32, kind="ExternalInput").ap()
    mal_d = nc.dram_tensor("mall", [P, (N // P) * QW], mask_dt,
                           kind="ExternalInput").ap()
    out_d = nc.dram_tensor("out", [QW, D], f32, kind="ExternalOutput").ap()

    with tile.TileContext(nc) as tc:
        with (
            tc.tile_pool(name="const", bufs=1) as cp,
            tc.tile_pool(name="khead", bufs=2) as kp,
            tc.tile_pool(name="vhead", bufs=2) as vp,
            tc.tile_pool(name="maskp", bufs=1) as mp,
            tc.tile_pool(name="qhead", bufs=2) as qp,
            tc.tile_pool(name="work", bufs=4) as wp,
            tc.tile_pool(name="accs", bufs=1) as ac,
            tc.tile_pool(name="ln", bufs=2) as lp,
            tc.tile_pool(name="psA", bufs=4, space="PSUM") as psA,
            tc.tile_pool(name="psO", bufs=1, space="PSUM") as psO,
            tc.tile_pool(name="dram", bufs=1, space="DRAM") as dp,
        ):
            # ---------- PE warmup: dummy matmuls on uninitialized SBUF so
            # the HAM clock-gate reaches K=8/8 while input DMAs stream in.
            wu = cp.tile([P, 640], mybir.dt.bfloat16, tag="wu")
            nc.vector.memset(wu[:], 0.125)
            wups = psA.tile([P, 512], f32, tag="ps", name="wups")
            for _ in range(40):
                nc.tensor.matmul(wups[:], lhsT=wu[:, :P], rhs=wu[:, P:P + 512],
                                 start=True, stop=True)

            # ---------- load inputs into SBUF ----------
            # DMA queue is FIFO: land the q-projection inputs first so the
            # first real matmuls start as early as possible.
            xq = cp.tile([P, 2 * QW], cdt, tag="xq")
            nc.sync.dma_start(out=xq[:].rearrange("p (i q) -> p i q", q=QW),
                              in_=xq_d[:].rearrange("(i p) q -> p i q", p=P))
            wq = cp.tile([P, H * 2 * D], cdt, tag="wq")
            wk = cp.tile([P, H * 2 * D], cdt, tag="wk")
            wv = cp.tile([P, H * 2 * D], cdt, tag="wv")
            for wsb, wd in ((wq, wq_d), (wk, wk_d), (wv, wv_d)):
                nc.sync.dma_start(
                    out=wsb[:].rearrange("p (h i d) -> p h i d", h=H, i=2),
                    in_=wd[:].rearrange("h (i p) d -> p h i d", p=P))
            xT = cp.tile([P, 2 * N], cdt, tag="xT")
            NQ = N // 4
            for q4 in range(4):
                nc.sync.dma_start(
                    out=xT[:].rearrange("p (i n) -> p i n", n=N)
                        [:, :, q4 * NQ:(q4 + 1) * NQ],
                    in_=xT_d[:].rearrange("(i p) n -> p i n", p=P)
                        [:, :, q4 * NQ:(q4 + 1) * NQ])
            gam = cp.tile([P, D], f32, tag="gam")
            bet = cp.tile([P, D], f32, tag="bet")
            bia = cp.tile([P, D], f32, tag="bia")
            nc.sync.dma_start(out=gam[:], in_=gam_d[:])
            nc.sync.dma_start(out=bet[:], in_=bet_d[:])
            nc.sync.dma_start(out=bia[:], in_=bia_d[:])
            epsc = cp.tile([P, 1], f32, tag="epsc")
            nc.gpsimd.memset(epsc[:], EPS)
            eps2 = cp.tile([P, 1], f32, tag="eps2")
            nc.gpsimd.memset(eps2[:], float(D) * float(D) * EPS)
            onescol = cp.tile([P, 2 * KV], f32, tag="onescol")
            nc.gpsimd.memset(onescol[:], 1.0)
            sqwarm = cp.tile([P, 1], f32, tag="sqwarm")
            nc.scalar.activation(sqwarm[:], epsc[:], Sqrt, bias=epsc[:])

            # ---------- edge-mask stripe (host-sharded input) to SBUF ----
            # quarters: issued after inputs on the same queue; attention
            # chunk c waits only for its quarter
            Mall = mp.tile([P, KV * QW], mask_dt, tag="mask")
            MQ = KV // 4
            for q4 in range(4):
                nc.sync.dma_start(
                    out=Mall[:, q4 * MQ * QW:(q4 + 1) * MQ * QW],
                    in_=mal_d[:, q4 * MQ * QW:(q4 + 1) * MQ * QW])

            # ---------- per-head compute ----------
            acc = [ac.tile([P, D], f32, tag=f"acc{s}", name=f"acc{s}")
                   for s in range(QS)]

            def make_proj(h):
                """Allocate head-h tiles; return (tiles, emit-thunks).

                Each thunk emits one PSUM matmul pair + its psum->sbuf copy;
                thunks are interleaved into the previous head's attention so
                the copies spread over a window where DVE/ACT have slack."""
                qT = qp.tile([P, 2 * QW], cdt, tag="qT", name=f"qT{h}")
                kT = kp.tile([P, 2 * N], cdt, tag="kT", name=f"kT{h}")
                vE = vp.tile([P, KV * D1], cdt, tag="vE", name=f"vE{h}")
                ops = []
                eng = [0]

                def qT_pair(j):
                    ps = psA.tile([P, 512], f32, tag="ps", name=f"q{h}_{j}")
                    for i in range(2):
                        w = (h * 2 + i) * D + j * P
                        nc.tensor.matmul(ps[:, :QW], lhsT=wq[:, w:w + P],
                                         rhs=xq[:, i * QW:(i + 1) * QW],
                                         start=(i == 0), stop=(i == 1))
                    nc.vector.tensor_copy(qT[:, j * QW:(j + 1) * QW],
                                          ps[:, :QW])

                def kT_pair(b, j):
                    ps = psA.tile([P, 512], f32, tag="ps", name=f"k{h}_{b}_{j}")
                    for i in range(2):
                        w = (h * 2 + i) * D + j * P
                        nc.tensor.matmul(
                            ps[:], lhsT=wk[:, w:w + P],
                            rhs=xT[:, i * N + b * 512:i * N + (b + 1) * 512],
                            start=(i == 0), stop=(i == 1))
                    dst = kT[:, j * N + b * 512:j * N + (b + 1) * 512]
                    if eng[0] % 2 == 0:
                        nc.scalar.copy(dst, ps[:])
                    else:
                        nc.vector.tensor_copy(dst, ps[:])
                    eng[0] += 1

                def vE_pair(c):
                    ps = psA.tile([P, 512], f32, tag="ps", name=f"v{h}_{c}")
                    for i in range(2):
                        nc.tensor.matmul(
                            ps[:, :D],
                            lhsT=xT[:, i * N + c * P:i * N + c * P + P],
                            rhs=wv[:, (h * 2 + i) * D:(h * 2 + i + 1) * D],
                            start=(i == 0), stop=(i == 1))
                    # head 0 runs upfront with ACT otherwise idle (no exp yet):
                    # split its copies across both engines; later heads keep
                    # vE on DVE so ACT has headroom for the interleaved exp
                    if h == 0 and c % 2 == 0:
                        nc.scalar.copy(vE[:, c * D1:c * D1 + D], ps[:, :D])
                    else:
                        nc.vector.tensor_copy(vE[:, c * D1:c * D1 + D],
                                              ps[:, :D])

                def ones_cols():
                    nc.vector.tensor_copy(
                        vE[:].rearrange("p (c e) -> p c e", e=D1)[:, :, D:D + 2],
                        onescol[:].rearrange("p (c e) -> p c e", e=2))

                for j in range(2):
                    ops.append(lambda j=j: qT_pair(j))
                for b in range(NB):
                    for j in range(2):
                        ops.append(lambda b=b, j=j: kT_pair(b, j))
                for c in range(KV):
                    ops.append(lambda c=c: vE_pair(c))
                ops.append(ones_cols)
                return (qT, kT, vE), ops

            cur, ops0 = make_proj(0)
            for op in ops0:
                op()

            for h in range(H):
                qT, kT, vE = cur
                if h + 1 < H:
                    nxt, pend = make_proj(h + 1)
                else:
                    nxt, pend = None, []
                per_chunk = -(-len(pend) // (KV - 4)) if pend else 0

                oPS = [psO.tile([P, D1], f32, tag=f"oPS{s}", name=f"oPS{s}")
                       for s in range(QS)]

                def o_mms(c, et):
                    for s in range(QS):
                        nc.tensor.matmul(oPS[s][:],
                                         lhsT=et[:, s * P:(s + 1) * P],
                                         rhs=vE[:, c * D1:(c + 1) * D1],
                                         start=(c == 0), stop=(c == KV - 1))

                prev = None
                for c in range(KV):
                    sc = psA.tile([P, 512], f32, tag="ps")
                    for j in range(2):
                        nc.tensor.matmul(sc[:, :QW],
                                         lhsT=kT[:, j * N + c * P:j * N + c * P + P],
                                         rhs=qT[:, j * QW:(j + 1) * QW],
                                         start=(j == 0), stop=(j == 1))
                    et = wp.tile([P, QW], cdt, tag="et")
                    nc.scalar.activation(et[:], sc[:, :QW], Exp, scale=1.0 / 16.0)
                    nc.vector.tensor_mul(et[:], et[:], Mall[:, c * QW:(c + 1) * QW])
                    if prev is not None:
                        o_mms(*prev)
                    prev = (c, et)
                    if c >= 4:
                        for _ in range(per_chunk):
                            if pend:
                                pend.pop(0)()
                o_mms(*prev)
                while pend:
                    pend.pop(0)()

                for s in range(QS):
                    rec = lp.tile([P, 1], f32, tag="rec")
                    nc.vector.reciprocal(rec[:], oPS[s][:, D:D + 1])
                    if h == 0:
                        nc.vector.tensor_scalar_mul(acc[s][:], oPS[s][:, 0:D], rec[:])
                    else:
                        tmp = lp.tile([P, D], f32, tag="tmp")
                        nc.vector.tensor_scalar_mul(tmp[:], oPS[s][:, 0:D], rec[:])
                        nc.vector.tensor_add(acc[s][:], acc[s][:], tmp[:])
                cur = nxt

            # ---------- bias + LayerNorm + store ----------
            inv_d = 1.0 / D
            Square = mybir.ActivationFunctionType.Square
            for s in range(QS):
                t = acc[s]
                if not triv_bias:
                    nc.vector.tensor_add(t[:], t[:], bia[:])
                musum = lp.tile([P, 1], f32, tag="musum")
                nc.vector.reduce_sum(musum[:], t[:], axis=AX)
                # LN is scale-invariant: center as D*t - sum(t), compensate in
                # the sqrt (scale 1/D, bias D^2*eps) -- one op fewer per slice
                xc = lp.tile([P, D], f32, tag="xc")
                nc.vector.tensor_scalar(out=xc[:], in0=t[:], scalar1=float(D),
                                        scalar2=musum[:],
                                        op0=MUL, op1=mybir.AluOpType.subtract)
                sq = lp.tile([P, D], f32, tag="sq")
                vs = lp.tile([P, 1], f32, tag="vs")
                nc.scalar.activation(sq[:], xc[:], Square, accum_out=vs[:])
                sd = lp.tile([P, 1], f32, tag="sd")
                nc.scalar.activation(sd[:], vs[:], Sqrt, bias=eps2[:], scale=inv_d)
                rs = lp.tile([P, 1], f32, tag="rs")
                nc.vector.reciprocal(rs[:], sd[:])
                og = lp.tile([P, D], f32, tag="og")
                if triv_gamma:
                    nc.vector.tensor_scalar_mul(og[:], xc[:], rs[:])
                else:
                    nc.vector.scalar_tensor_tensor(og[:], in0=xc[:],
                                                   scalar=rs[:], in1=gam[:],
                                                   op0=MUL, op1=MUL)
                if triv_beta:
                    nc.sync.dma_start(out=out_d[s * P:(s + 1) * P, :], in_=og[:])
                else:
                    oo = lp.tile([P, D], f32, tag="oo")
                    nc.vector.tensor_add(oo[:], og[:], bet[:])
                    nc.sync.dma_start(out=out_d[s * P:(s + 1) * P, :], in_=oo[:])

    nc.compile()
    return nc


def _prep_host(inputs, N, QW):
    """Host-side input resharding: transposes, folded weights, mask offsets."""
    x = np.ascontiguousarray(np.asarray(inputs["x"], dtype=np.float32))
    ei = np.asarray(inputs["edge_index"]).astype(np.int64)
    Wq = np.asarray(inputs["Wq"], dtype=np.float64)
    Wk = np.asarray(inputs["Wk"], dtype=np.float64)
    Wv = np.asarray(inputs["Wv"], dtype=np.float64)
    Wo = np.asarray(inputs["Wo"], dtype=np.float64)
    Wp = np.asarray(inputs["Wp"], dtype=np.float64)
    bq = np.asarray(inputs["bq"], dtype=np.float64)
    bk = np.asarray(inputs["bk"], dtype=np.float64)
    bv = np.asarray(inputs["bv"], dtype=np.float64)
    bo = np.asarray(inputs["bo"], dtype=np.float64)
    bp = np.asarray(inputs["bp"], dtype=np.float64)
    gamma = np.asarray(inputs["gamma"], dtype=np.float32)
    beta = np.asarray(inputs["beta"], dtype=np.float32)

    assert not bq.any() and not bk.any(), \
        "nonzero q/k biases not wired in the device graph"

    xT = np.ascontiguousarray(x.T)                       # [D, N]
    wq_h = np.ascontiguousarray(
        np.stack([Wq[h].T for h in range(H)]).astype(np.float32))
    wk_h = np.ascontiguousarray(
        np.stack([Wk[h].T for h in range(H)]).astype(np.float32))
    # folded v' weight and total bias
    wv_l, bias_tot = [], bp.copy()
    for h in range(H):
        Wp_h = Wp[:, h * D:(h + 1) * D]                  # [f, e']
        G = Wo[h].T @ Wp_h.T                             # [e, f]
        wv_l.append(Wv[h].T @ G)                         # [d, f]
        bias_tot = bias_tot + bo[h] @ Wp_h.T + bv[h] @ G
    wv_h = np.ascontiguousarray(np.stack(wv_l).astype(np.float32))

    gam_b = np.ascontiguousarray(np.broadcast_to(gamma, (P, D)).astype(np.float32))
    bet_b = np.ascontiguousarray(np.broadcast_to(beta, (P, D)).astype(np.float32))
    bia_b = np.ascontiguousarray(
        np.broadcast_to(bias_tot.astype(np.float32), (P, D)))

    # mask stripes per core, pre-arranged to the SBUF layout
    # mall[p, c*QW + q] = adjacency[c*P + p, q0 + q]  (kv-major, symmetric+diag)
    import ml_dtypes
    adj = np.zeros((N, N), dtype=np.uint8)
    r, c = ei[0], ei[1]
    adj[r, c] = 1
    adj[c, r] = 1
    adj[np.arange(N), np.arange(N)] = 1
    KV = N // P
    malls = []
    for core in range(N_CORES):
        q0 = core * QW
        stripe = adj[:, q0:q0 + QW]                      # [N(kv), QW]
        m = stripe.reshape(KV, P, QW).transpose(1, 0, 2).reshape(P, KV * QW)
        malls.append(np.ascontiguousarray(m.astype(ml_dtypes.bfloat16)))
    return xT, wq_h, wk_h, wv_h, gam_b, bet_b, bia_b, malls


def _run(inputs, trace=False, mask_dt_name="bfloat16", mode="f32r",
         tmpdir=None):
    from concourse.bass_utils import run_bass_kernel_spmd
    from concourse.bass_interp import get_hw_module

    N = int(np.asarray(inputs["x"]).shape[0])
    QW = N // N_CORES
    (xT, wq_h, wk_h, wv_h, gam_b, bet_b, bia_b, malls) = \
        _prep_host(inputs, N, QW)

    if mode == "bf16":
        import ml_dtypes
        hdt = ml_dtypes.bfloat16
        xT = xT.astype(hdt)
        wq_h, wk_h, wv_h = (a.astype(hdt) for a in (wq_h, wk_h, wv_h))
    elif mode == "f32r":
        # fp32r operands must be pre-rounded (RNE dropping 12 mantissa bits);
        # matches walrus fp32_to_fp32r.
        def _r(a):
            b = a.view(np.uint32).astype(np.uint64)
            rb = (b + 0x7FF + ((b >> 12) & 1)) & np.uint64(0xFFFFF000)
            return rb.astype(np.uint32).view(np.float32)
        xT = _r(xT)
        wq_h, wk_h, wv_h = _r(wq_h), _r(wk_h), _r(wv_h)
    gamma = np.asarray(inputs["gamma"], np.float64)
    beta = np.asarray(inputs["beta"], np.float64)
    key = (N, QW, mask_dt_name, mode, not np.any(bia_b),
           bool((gamma == 1).all()), not beta.any())
    nc = _BUILD_CACHE.get(key)
    if nc is None:
        nc = _build(N, QW, mask_dt_name=mask_dt_name, mode=mode,
                    triv_bias=key[4], triv_gamma=key[5], triv_beta=key[6])
        _BUILD_CACHE[key] = nc
    old = nc.m
    nc.m = get_hw_module(nc.m)
    try:
        in_maps = []
        for core in range(N_CORES):
            q0 = core * QW
            in_maps.append({
                "xT": xT,
                "xq": np.ascontiguousarray(xT[:, q0:q0 + QW]),
                "wq": wq_h, "wk": wk_h, "wv": wv_h,
                "gamma_b": gam_b, "beta_b": bet_b, "bias_b": bia_b,
                "mall": malls[core],
            })
        res = run_bass_kernel_spmd(nc, in_maps, core_ids=list(range(N_CORES)),
                                   trace=trace, tmpdir=tmpdir)
    finally:
        nc.m = old
    out = np.concatenate([res.results[i]["out"] for i in range(N_CORES)], axis=0)
    return out.astype(np.float32), res


def kernel(**inputs) -> np.ndarray:
    out, _ = _run(inputs)
    return out


def _build2(N, mask_dt_name="bfloat16", mode="bf16"):
    """Hybrid sharding: core c owns head (c>>1) and row-half (c&1).

    Projections are per-head only (1/4 the replicated work of _build); the
    per-head partial outputs are summed across the 4 cores of each half via
    ReduceScatter (groups [[0,2,4,6],[1,3,5,7]]), one RS per 512-row
    superblock so all but the last overlap attention compute.  Core c's
    RS shard sb covers global rows (c&1)*2048 + sb*512 + (c>>1)*128.
    """
    import concourse.bacc as bacc
    import concourse.tile as tile
    import concourse.bass as bass
    from concourse import mybir

    f32 = mybir.dt.float32
    mask_dt = getattr(mybir.dt, mask_dt_name)
    cdt = {"f32r": mybir.dt.float32r, "bf16": mybir.dt.bfloat16,
           "f32": f32}[mode]
    Exp = mybir.ActivationFunctionType.Exp
    Copy = mybir.ActivationFunctionType.Copy
    Sqrt = mybir.ActivationFunctionType.Sqrt
    AX = mybir.AxisListType.X
    MUL = mybir.AluOpType.mult
    KV = N // P                 # kv chunks of 128
    HW_ = N // 2                # half-window width (2048)
    SBW = 512                   # superblock width
    NSB = HW_ // SBW            # superblocks (4)
    NB = N // 512
    D1 = D + 2

    nc = bacc.Bacc("TRN2", target_bir_lowering=False, debug=False,
                   num_devices=N_CORES)

    xT_d = nc.dram_tensor("xT", [D, N], cdt, kind="ExternalInput").ap()
    xq_d = nc.dram_tensor("xq", [D, HW_], cdt, kind="ExternalInput").ap()
    wq_d = nc.dram_tensor("wq", [D, D], cdt, kind="ExternalInput").ap()
    wk_d = nc.dram_tensor("wk", [D, D], cdt, kind="ExternalInput").ap()
    wv_d = nc.dram_tensor("wv", [D, D], cdt, kind="ExternalInput").ap()
    gam_d = nc.dram_tensor("gamma_b", [P, D], f32, kind="ExternalInput").ap()
    bet_d = nc.dram_tensor("beta_b", [P, D], f32, kind="ExternalInput").ap()
    bia_d = nc.dram_tensor("bias_b", [P, D], f32, kind="ExternalInput").ap()
    mal_d = nc.dram_tensor("mall", [P, NSB * KV * SBW], mask_dt,
                           kind="ExternalInput").ap()
    out_d = nc.dram_tensor("out", [NSB * P, D], f32, kind="ExternalOutput").ap()

    groups = [[0, 2, 4, 6], [1, 3, 5, 7]]

    with tile.TileContext(nc) as tc:
        with (
            tc.tile_pool(name="const", bufs=1) as cp,
            tc.tile_pool(name="maskp", bufs=2) as mp,
            tc.tile_pool(name="work", bufs=4) as wp,
            tc.tile_pool(name="accs", bufs=1) as ac,
            tc.tile_pool(name="ln", bufs=2) as lp,
            tc.tile_pool(name="psA", bufs=3, space="PSUM") as psA,
            tc.tile_pool(name="psO", bufs=1, space="PSUM") as psO,
            tc.tile_pool(name="dram", bufs=1, space="DRAM") as dp,
        ):
            wu = cp.tile([P, 640], mybir.dt.bfloat16, tag="wu")
            nc.vector.memset(wu[:], 0.125)
            wups = psA.tile([P, 512], f32, tag="ps", name="wups")
            for _ in range(16):
                nc.tensor.matmul(wups[:], lhsT=wu[:, :P], rhs=wu[:, P:P + 512],
                                 start=True, stop=True)

            xq = cp.tile([P, 2 * HW_], cdt, tag="xq")
            nc.sync.dma_start(out=xq[:].rearrange("p (i q) -> p i q", q=HW_),
                              in_=xq_d[:].rearrange("(i p) q -> p i q", p=P))
            wq = cp.tile([P, 2 * D], cdt, tag="wq")
            wk = cp.tile([P, 2 * D], cdt, tag="wk")
            wv = cp.tile([P, 2 * D], cdt, tag="wv")
            for wsb, wd in ((wq, wq_d), (wk, wk_d), (wv, wv_d)):
                nc.sync.dma_start(
                    out=wsb[:].rearrange("p (i d) -> p i d", i=2),
                    in_=wd[:].rearrange("(i p) d -> p i d", p=P))
            xT = cp.tile([P, 2 * N], cdt, tag="xT")
            nc.sync.dma_start(out=xT[:].rearrange("p (i n) -> p i n", n=N),
                              in_=xT_d[:].rearrange("(i p) n -> p i n", p=P))
            gam = cp.tile([P, D], f32, tag="gam")
            bet = cp.tile([P, D], f32, tag="bet")
            bia = cp.tile([P, D], f32, tag="bia")
            nc.sync.dma_start(out=gam[:], in_=gam_d[:])
            nc.sync.dma_start(out=bet[:], in_=bet_d[:])
            nc.sync.dma_start(out=bia[:], in_=bia_d[:])
            epsc = cp.tile([P, 1], f32, tag="epsc")
            nc.gpsimd.memset(epsc[:], EPS)
            onescol = cp.tile([P, 2 * KV], f32, tag="onescol")
            nc.gpsimd.memset(onescol[:], 1.0)

            # projections (single head)
            qT = cp.tile([P, 2 * HW_], cdt, tag="qT")
            for j in range(2):
                for qb in range(HW_ // 512):
                    ps = psA.tile([P, 512], f32, tag="ps")
                    for i in range(2):
                        nc.tensor.matmul(
                            ps[:],
                            lhsT=wq[:, i * D + j * P:i * D + j * P + P],
                            rhs=xq[:, i * HW_ + qb * 512:i * HW_ + (qb + 1) * 512],
                            start=(i == 0), stop=(i == 1))
                    nc.vector.tensor_copy(
                        qT[:, j * HW_ + qb * 512:j * HW_ + (qb + 1) * 512], ps[:])
            kT = cp.tile([P, 2 * N], cdt, tag="kT")
            for j in range(2):
                for b in range(NB):
                    ps = psA.tile([P, 512], f32, tag="ps")
                    for i in range(2):
                        nc.tensor.matmul(
                            ps[:],
                            lhsT=wk[:, i * D + j * P:i * D + j * P + P],
                            rhs=xT[:, i * N + b * 512:i * N + (b + 1) * 512],
                            start=(i == 0), stop=(i == 1))
                    if b % 2 == 0:
                        nc.scalar.copy(
                            kT[:, j * N + b * 512:j * N + (b + 1) * 512], ps[:])
                    else:
                        nc.vector.tensor_copy(
                            kT[:, j * N + b * 512:j * N + (b + 1) * 512], ps[:])
            vE = cp.tile([P, KV * D1], cdt, tag="vE")
            for c in range(KV):
                ps = psA.tile([P, 512], f32, tag="ps")
                for i in range(2):
                    nc.tensor.matmul(
                        ps[:, :D],
                        lhsT=xT[:, i * N + c * P:i * N + c * P + P],
                        rhs=wv[:, i * D:(i + 1) * D],
                        start=(i == 0), stop=(i == 1))
                nc.vector.tensor_copy(vE[:, c * D1:c * D1 + D], ps[:, :D])
            nc.vector.tensor_copy(
                vE[:].rearrange("p (c e) -> p c e", e=D1)[:, :, D:D + 2],
                onescol[:].rearrange("p (c e) -> p c e", e=2))

            # RS bounce buffers (one collective at the end)
            bounce_in = dp.tile([NSB * 4 * P, D], f32, name="bin")
            bounce_out = dp.tile([NSB * P, D], f32, name="bout")

            for sb in range(NSB):
                Msb = mp.tile([P, KV * SBW], mask_dt, tag="Msb",
                              name=f"Msb{sb}")
                for c in range(KV):
                    nc.sync.dma_start(
                        out=Msb[:, c * SBW:(c + 1) * SBW],
                        in_=mal_d[:, (sb * KV + c) * SBW:(sb * KV + c + 1) * SBW])
                oPS = [psO.tile([P, D1], f32, tag=f"oPS{s}", name=f"oPS{sb}_{s}")
                       for s in range(4)]

                def o_mms(c, et):
                    for s in range(4):
                        nc.tensor.matmul(oPS[s][:],
                                         lhsT=et[:, s * P:(s + 1) * P],
                                         rhs=vE[:, c * D1:(c + 1) * D1],
                                         start=(c == 0), stop=(c == KV - 1))

                prev = None
                for c in range(KV):
                    sc = psA.tile([P, 512], f32, tag="ps", name=f"sc{sb}_{c}")
                    for j in range(2):
                        nc.tensor.matmul(
                            sc[:],
                            lhsT=kT[:, j * N + c * P:j * N + c * P + P],
                            rhs=qT[:, j * HW_ + sb * SBW:j * HW_ + (sb + 1) * SBW],
                            start=(j == 0), stop=(j == 1))
                    et = wp.tile([P, SBW], cdt, tag="et", name=f"et{sb}_{c}")
                    nc.scalar.activation(et[:], sc[:], Exp, scale=1.0 / 16.0)
                    nc.vector.tensor_mul(et[:], et[:],
                                         Msb[:, c * SBW:(c + 1) * SBW])
                    if prev is not None:
                        o_mms(*prev)
                    prev = (c, et)
                o_mms(*prev)

                for s in range(4):
                    rec = lp.tile([P, 1], f32, tag="rec", name=f"rec{sb}_{s}")
                    nc.vector.reciprocal(rec[:], oPS[s][:, D:D + 1])
                    par = lp.tile([P, D], f32, tag="par", name=f"par{sb}_{s}")
                    nc.vector.tensor_scalar_mul(par[:], oPS[s][:, 0:D], rec[:])
                    nc.sync.dma_start(
                        out=bounce_in[(sb * 4 + s) * P:(sb * 4 + s + 1) * P, :],
                        in_=par[:])
            nc.gpsimd.collective_compute(
                "ReduceScatter", mybir.AluOpType.add,
                replica_groups=groups,
                ins=[bounce_in[:].opt()],
                outs=[bounce_out[:].opt()],
            )

            # post-RS: bias + LayerNorm + store, per superblock shard
            inv_d = 1.0 / D
            for sb in range(NSB):
                t = ac.tile([P, D], f32, tag="acc", name=f"post{sb}")
                nc.sync.dma_start(out=t[:],
                                  in_=bounce_out[sb * P:(sb + 1) * P, :])
                nc.vector.tensor_add(t[:], t[:], bia[:])
                musum = lp.tile([P, 1], f32, tag="musum", name=f"mus{sb}")
                nc.vector.reduce_sum(musum[:], t[:], axis=AX)
                mu = lp.tile([P, 1], f32, tag="mu", name=f"mu{sb}")
                nc.scalar.activation(mu[:], musum[:], Copy, scale=inv_d)
                xc = lp.tile([P, D], f32, tag="xc", name=f"xc{sb}")
                nc.vector.tensor_scalar_sub(xc[:], t[:], mu[:])
                sq = lp.tile([P, D], f32, tag="sq", name=f"sq{sb}")
                nc.vector.tensor_mul(sq[:], xc[:], xc[:])
                vs = lp.tile([P, 1], f32, tag="vs", name=f"vs{sb}")
                nc.vector.reduce_sum(vs[:], sq[:], axis=AX)
                sd = lp.tile([P, 1], f32, tag="sd", name=f"sd{sb}")
                nc.scalar.activation(sd[:], vs[:], Sqrt, bias=epsc[:],
                                     scale=inv_d)
                rs = lp.tile([P, 1], f32, tag="rs", name=f"rs{sb}")
                nc.vector.reciprocal(rs[:], sd[:])
                og = lp.tile([P, D], f32, tag="og", name=f"og{sb}")
                nc.vector.scalar_tensor_tensor(og[:], in0=xc[:], scalar=rs[:],
                                               in1=gam[:], op0=MUL, op1=MUL)
                oo = lp.tile([P, D], f32, tag="oo", name=f"oo{sb}")
                nc.vector.tensor_add(oo[:], og[:], bet[:])
                nc.sync.dma_start(out=out_d[sb * P:(sb + 1) * P, :], in_=oo[:])

    nc.compile()
    return nc


def _run2(inputs, trace=False, mask_dt_name="bfloat16", mode="bf16",
          tmpdir=None):
    from concourse.bass_utils import run_bass_kernel_spmd
    from concourse.bass_interp import get_hw_module
    import ml_dtypes

    N = int(np.asarray(inputs["x"]).shape[0])
    QW = N // N_CORES
    (xT, wq_h, wk_h, wv_h, gam_b, bet_b, bia_b, _malls) =         _prep_host(inputs, N, QW)
    ei = np.asarray(inputs["edge_index"]).astype(np.int64)
    adj = np.zeros((N, N), dtype=np.uint8)
    adj[ei[0], ei[1]] = 1
    adj[ei[1], ei[0]] = 1
    adj[np.arange(N), np.arange(N)] = 1

    if mode == "bf16":
        hdt = ml_dtypes.bfloat16
        xT = xT.astype(hdt)
        wq_h, wk_h, wv_h = (a.astype(hdt) for a in (wq_h, wk_h, wv_h))

    KV = N // P
    HW_ = N // 2
    SBW = 512
    NSB = HW_ // SBW
    nc = _build2(N, mask_dt_name=mask_dt_name, mode=mode)
    old = nc.m
    nc.m = get_hw_module(nc.m)
    try:
        in_maps = []
        for core in range(N_CORES):
            h, r = core >> 1, core & 1
            q0 = r * HW_
            stripe = adj[:, q0:q0 + HW_]
            m = np.zeros((P, NSB * KV * SBW), np.uint8)
            for sb in range(NSB):
                blk = stripe[:, sb * SBW:(sb + 1) * SBW]
                m[:, sb * KV * SBW:(sb + 1) * KV * SBW] = (
                    blk.reshape(KV, P, SBW).transpose(1, 0, 2)
                    .reshape(P, KV * SBW))
            in_maps.append({
                "xT": xT,
                "xq": np.ascontiguousarray(xT[:, q0:q0 + HW_]),
                "wq": wq_h[h], "wk": wk_h[h], "wv": wv_h[h],
                "gamma_b": gam_b, "beta_b": bet_b, "bias_b": bia_b,
                "mall": np.ascontiguousarray(m.astype(ml_dtypes.bfloat16)),
            })
        res = run_bass_kernel_spmd(nc, in_maps, core_ids=list(range(N_CORES)),
                                   trace=trace, tmpdir=tmpdir)
    finally:
        nc.m = old
    out = np.zeros((N, D), np.float32)
    for core in range(N_CORES):
        h, r = core >> 1, core & 1
        g0 = r * HW_ + h * (NSB * P)
        out[g0:g0 + NSB * P] = res.results[core]["out"]
    return out.astype(np.float32), res


# revision 37
# speedup vs baseline: 1.1887x; 1.1771x over previous
"""Trainium2 Bass kernel for nn_AdaptiveGraphConvLayer (graph multi-head attention).

Reference computation:
    mask = dense additive edge mask from edge_index (symmetric + self loops)
    per head h: q,k,v projections of x; scores = q @ k.T / 16 + mask; softmax
    o_h = attn @ v_h; head_out_h = o_h @ Wo_h.T + bo_h
    out = concat_h(head_out) @ Wp.T + bp;  LayerNorm(out) * gamma + beta
    (N=4096 nodes, D=256, H=4 heads, E=131072 edges; ~80 GFLOP)

Measured: ~210 us HW exec on 8 NeuronCores at full clock (chip P0 power
throttling, when active after sustained load, scales this ~1.2x), rel err
3.2e-3 (bf16 matmuls, fp32 accumulate/softmax/LayerNorm).

Device strategy (kernel(): node-parallel, zero collectives):
  - Core c owns query rows [c*512, (c+1)*512) for ALL 4 heads; k/v
    projections are recomputed per core.  On this setup a measured
    collective costs ~45-60 us (floor-dominated), more than the ~55 us of
    replicated projection matmuls it could remove, so the comm-free layout
    wins (a head-parallel + ReduceScatter variant, _build2/_run2, measured
    equal at best).
  - Algebraic fold: out = sum_h attn_h @ v'_h + bias_tot with
        v'_h = x @ (Wv_h^T (Wp_h Wo_h)^T)   (host-precomputed weight)
    which eliminates the per-head out-proj and final projection entirely.
  - scoresT blocks [kv=128, q=512] = kT-slices^T @ qT; exp on ACT
    (scale=1/16, no max-subtract needed: |scores| < ~1 and every row has a
    self loop); mask applied multiplicatively on DVE; softmax denominator
    via ones-columns appended to v' (o_ext[:, D] = row sum), normalized
    with a per-partition reciprocal.  o-matmuls run one kv-chunk behind the
    exp/mask pipeline so the PE never stalls.
  - Edge mask: host reshards edge_index into per-core dense {0,1} bf16
    stripes in SBUF layout (indirect-DMA scatter on real HW honors only one
    offset per partition per instruction, so an on-device build would cost
    ~260 serial SWDGE instructions ~ 300 us; host resharding keeps all
    FLOPs and all on-chip traffic on device).
  - bf16 everywhere on the PE (fast weight load; fp32r needs pre-rounded
    operands and loads weights 2x slower), fp32 PSUM accumulate, fp32
    softmax/normalize/LayerNorm.  psum->sbuf casts split across ACT/DVE.
  - Head h+1's projection matmul pairs (and their psum->sbuf copies) are
    emitted interleaved into head h's attention kv-loop, spreading the
    copy work over a window where DVE/ACT have slack -- this removed
    ~0.5-1.2 us copy-backpressure stalls per projection pair at head
    boundaries (-16 us total).
  - Prologue: 40 dummy warmup matmuls keep the PE HAM clock-gate at 8/8
    through the input-DMA window; inputs land via few merged strided DMAs
    (sync-queue issue costs ~0.65 us per DMA instruction).
  - Tail: fused Square+accum_out variance, Sqrt table preloaded, affine
    LN ops elided when gamma/beta/bias are trivial for the given inputs.
"""

import numpy as np

N_FULL = 4096
D = 256
H = 4
N_CORES = 8
EPS = 1e-5
P = 128  # partitions


def _build(N, QW, mask_dt_name="bfloat16", mode="f32r",
           triv_bias=False, triv_gamma=False, triv_beta=False):
    """Build + compile the SPMD Bass graph (identical on all cores)."""
    import concourse.bacc as bacc
    import concourse.tile as tile
    import concourse.bass as bass
    from concourse import mybir

    f32 = mybir.dt.float32
    i32 = mybir.dt.int32
    mask_dt = getattr(mybir.dt, mask_dt_name)
    cdt = {"f32r": mybir.dt.float32r, "bf16": mybir.dt.bfloat16,
           "f32": f32}[mode]
    Exp = mybir.ActivationFunctionType.Exp
    Copy = mybir.ActivationFunctionType.Copy
    Sqrt = mybir.ActivationFunctionType.Sqrt
    AX = mybir.AxisListType.X
    MUL = mybir.AluOpType.mult
    KV = N // P            # kv chunks of 128
    QS = QW // P           # q slices of 128 within this core's window
    NB = N // 512          # 512-wide node blocks (kT projection)
    D1 = D + 2             # v' + ones columns (padded even for fp32r)

    def mc(ap):
        return ap

    nc = bacc.Bacc("TRN2", target_bir_lowering=False, debug=False,
                   num_devices=N_CORES)

    xT_d = nc.dram_tensor("xT", [D, N], cdt, kind="ExternalInput").ap()
    xq_d = nc.dram_tensor("xq", [D, QW], cdt, kind="ExternalInput").ap()
    wq_d = nc.dram_tensor("wq", [H, D, D], cdt, kind="ExternalInput").ap()
    wk_d = nc.dram_tensor("wk", [H, D, D], cdt, kind="ExternalInput").ap()
    wv_d = nc.dram_tensor("wv", [H, D, D], cdt, kind="ExternalInput").ap()
    gam_d = nc.dram_tensor("gamma_b", [P, D], f32, kind="ExternalInput").ap()
    bet_d = nc.dram_tensor("beta_b", [P, D], f32, kind="ExternalInput").ap()
    bia_d = nc.dram_tensor("bias_b", [P, D], f32, kind="ExternalInput").ap()
    mal_d = nc.dram_tensor("mall", [P, (N // P) * QW], mask_dt,
                           kind="ExternalInput").ap()
    out_d = nc.dram_tensor("out", [QW, D], f32, kind="ExternalOutput").ap()

    with tile.TileContext(nc) as tc:
        with (
            tc.tile_pool(name="const", bufs=1) as cp,
            tc.tile_pool(name="khead", bufs=2) as kp,
            tc.tile_pool(name="vhead", bufs=2) as vp,
            tc.tile_pool(name="maskp", bufs=1) as mp,
            tc.tile_pool(name="qhead", bufs=2) as qp,
            tc.tile_pool(name="work", bufs=4) as wp,
            tc.tile_pool(name="accs", bufs=1) as ac,
            tc.tile_pool(name="ln", bufs=2) as lp,
            tc.tile_pool(name="psA", bufs=4, space="PSUM") as psA,
            tc.tile_pool(name="psO", bufs=1, space="PSUM") as psO,
            tc.tile_pool(name="dram", bufs=1, space="DRAM") as dp,
        ):
            # ---------- PE warmup: dummy matmuls on uninitialized SBUF so
            # the HAM clock-gate reaches K=8/8 while input DMAs stream in.
            wu = cp.tile([P, 640], mybir.dt.bfloat16, tag="wu")
            nc.vector.memset(wu[:], 0.125)
            wups = psA.tile([P, 512], f32, tag="ps", name="wups")
            for _ in range(28):
                nc.tensor.matmul(wups[:], lhsT=wu[:, :P], rhs=wu[:, P:P + 512],
                                 start=True, stop=True)

            # ---------- load inputs into SBUF ----------
            # DMA queue is FIFO: land the q-projection inputs first so the
            # first real matmuls start as early as possible.
            xq = cp.tile([P, 2 * QW], cdt, tag="xq")
            nc.sync.dma_start(out=xq[:].rearrange("p (i q) -> p i q", q=QW),
                              in_=xq_d[:].rearrange("(i p) q -> p i q", p=P))
            wq = cp.tile([P, H * 2 * D], cdt, tag="wq")
            wk = cp.tile([P, H * 2 * D], cdt, tag="wk")
            wv = cp.tile([P, H * 2 * D], cdt, tag="wv")
            for wsb, wd in ((wq, wq_d), (wk, wk_d), (wv, wv_d)):
                nc.sync.dma_start(
                    out=wsb[:].rearrange("p (h i d) -> p h i d", h=H, i=2),
                    in_=wd[:].rearrange("h (i p) d -> p h i d", p=P))
            xT = cp.tile([P, 2 * N], cdt, tag="xT")
            NQ = N // 4
            for q4 in range(4):
                nc.sync.dma_start(
                    out=xT[:].rearrange("p (i n) -> p i n", n=N)
                        [:, :, q4 * NQ:(q4 + 1) * NQ],
                    in_=xT_d[:].rearrange("(i p) n -> p i n", p=P)
                        [:, :, q4 * NQ:(q4 + 1) * NQ])
            gam = cp.tile([P, D], f32, tag="gam")
            bet = cp.tile([P, D], f32, tag="bet")
            bia = cp.tile([P, D], f32, tag="bia")
            nc.sync.dma_start(out=gam[:], in_=gam_d[:])
            nc.sync.dma_start(out=bet[:], in_=bet_d[:])
            nc.sync.dma_start(out=bia[:], in_=bia_d[:])
            epsc = cp.tile([P, 1], f32, tag="epsc")
            nc.gpsimd.memset(epsc[:], EPS)
            eps2 = cp.tile([P, 1], f32, tag="eps2")
            nc.gpsimd.memset(eps2[:], float(D) * float(D) * EPS)
            onescol = cp.tile([P, 2 * KV], f32, tag="onescol")
            nc.gpsimd.memset(onescol[:], 1.0)
            sqwarm = cp.tile([P, 1], f32, tag="sqwarm")
            nc.scalar.activation(sqwarm[:], epsc[:], Sqrt, bias=epsc[:])

            # ---------- edge-mask stripe (host-sharded input) to SBUF ----
            # quarters: issued after inputs on the same queue; attention
            # chunk c waits only for its quarter
            Mall = mp.tile([P, KV * QW], mask_dt, tag="mask")
            MQ = KV // 4
            for q4 in range(4):
                nc.sync.dma_start(
                    out=Mall[:, q4 * MQ * QW:(q4 + 1) * MQ * QW],
                    in_=mal_d[:, q4 * MQ * QW:(q4 + 1) * MQ * QW])

            # ---------- per-head compute ----------
            acc = [ac.tile([P, D], f32, tag=f"acc{s}", name=f"acc{s}")
                   for s in range(QS)]

            def make_proj(h):
                """Allocate head-h tiles; return (tiles, emit-thunks).

                Each thunk emits one PSUM matmul pair + its psum->sbuf copy;
                thunks are interleaved into the previous head's attention so
                the copies spread over a window where DVE/ACT have slack."""
                qT = qp.tile([P, 2 * QW], cdt, tag="qT", name=f"qT{h}")
                kT = kp.tile([P, 2 * N], cdt, tag="kT", name=f"kT{h}")
                vE = vp.tile([P, KV * D1], cdt, tag="vE", name=f"vE{h}")
                ops = []
                eng = [0]

                def qT_pair(j):
                    ps = psA.tile([P, 512], f32, tag="ps", name=f"q{h}_{j}")
                    for i in range(2):
                        w = (h * 2 + i) * D + j * P
                        nc.tensor.matmul(ps[:, :QW], lhsT=wq[:, w:w + P],
                                         rhs=xq[:, i * QW:(i + 1) * QW],
                                         start=(i == 0), stop=(i == 1))
                    nc.vector.tensor_copy(qT[:, j * QW:(j + 1) * QW],
                                          ps[:, :QW])

                def kT_pair(b, j):
                    ps = psA.tile([P, 512], f32, tag="ps", name=f"k{h}_{b}_{j}")
                    for i in range(2):
                        w = (h * 2 + i) * D + j * P
                        nc.tensor.matmul(
                            ps[:], lhsT=wk[:, w:w + P],
                            rhs=xT[:, i * N + b * 512:i * N + (b + 1) * 512],
                            start=(i == 0), stop=(i == 1))
                    dst = kT[:, j * N + b * 512:j * N + (b + 1) * 512]
                    if eng[0] % 2 == 0:
                        nc.scalar.copy(dst, ps[:])
                    else:
                        nc.vector.tensor_copy(dst, ps[:])
                    eng[0] += 1

                def vE_pair(c):
                    ps = psA.tile([P, 512], f32, tag="ps", name=f"v{h}_{c}")
                    for i in range(2):
                        nc.tensor.matmul(
                            ps[:, :D],
                            lhsT=xT[:, i * N + c * P:i * N + c * P + P],
                            rhs=wv[:, (h * 2 + i) * D:(h * 2 + i + 1) * D],
                            start=(i == 0), stop=(i == 1))
                    # head 0 runs upfront with ACT otherwise idle (no exp yet):
                    # split its copies across both engines; later heads keep
                    # vE on DVE so ACT has headroom for the interleaved exp
                    if h == 0 and c % 2 == 0:
                        nc.scalar.copy(vE[:, c * D1:c * D1 + D], ps[:, :D])
                    else:
                        nc.vector.tensor_copy(vE[:, c * D1:c * D1 + D],
                                              ps[:, :D])

                def ones_cols():
                    nc.vector.tensor_copy(
                        vE[:].rearrange("p (c e) -> p c e", e=D1)[:, :, D:D + 2],
                        onescol[:].rearrange("p (c e) -> p c e", e=2))

                for j in range(2):
                    ops.append(lambda j=j: qT_pair(j))
                for b in range(NB):
                    for j in range(2):
                        ops.append(lambda b=b, j=j: kT_pair(b, j))
                for c in range(KV):
                    ops.append(lambda c=c: vE_pair(c))
                ops.append(ones_cols)
                return (qT, kT, vE), ops

            cur, ops0 = make_proj(0)
            for op in ops0:
                op()

            for h in range(H):
                qT, kT, vE = cur
                if h + 1 < H:
                    nxt, pend = make_proj(h + 1)
                else:
                    nxt, pend = None, []
                per_chunk = -(-len(pend) // (KV - 2)) if pend else 0

                oPS = [psO.tile([P, D1], f32, tag=f"oPS{s}", name=f"oPS{s}")
                       for s in range(QS)]

                def o_mms(c, et):
                    for s in range(QS):
                        nc.tensor.matmul(oPS[s][:],
                                         lhsT=et[:, s * P:(s + 1) * P],
                                         rhs=vE[:, c * D1:(c + 1) * D1],
                                         start=(c == 0), stop=(c == KV - 1))

                prev = None
                for c in range(KV):
                    sc = psA.tile([P, 512], f32, tag="ps")
                    for j in range(2):
                        nc.tensor.matmul(sc[:, :QW],
                                         lhsT=kT[:, j * N + c * P:j * N + c * P + P],
                                         rhs=qT[:, j * QW:(j + 1) * QW],
                                         start=(j == 0), stop=(j == 1))
                    et = wp.tile([P, QW], cdt, tag="et")
                    nc.scalar.activation(et[:], sc[:, :QW], Exp, scale=1.0 / 16.0)
                    nc.vector.tensor_mul(et[:], et[:], Mall[:, c * QW:(c + 1) * QW])
                    if prev is not None:
                        o_mms(*prev)
                    prev = (c, et)
                    if c >= 2:
                        for _ in range(per_chunk):
                            if pend:
                                pend.pop(0)()
                o_mms(*prev)
                while pend:
                    pend.pop(0)()

                for s in range(QS):
                    rec = lp.tile([P, 1], f32, tag="rec")
                    nc.vector.reciprocal(rec[:], oPS[s][:, D:D + 1])
                    if h == 0:
                        nc.vector.tensor_scalar_mul(acc[s][:], oPS[s][:, 0:D], rec[:])
                    else:
                        tmp = lp.tile([P, D], f32, tag="tmp")
                        nc.vector.tensor_scalar_mul(tmp[:], oPS[s][:, 0:D], rec[:])
                        nc.vector.tensor_add(acc[s][:], acc[s][:], tmp[:])
                cur = nxt

            # ---------- bias + LayerNorm + store ----------
            inv_d = 1.0 / D
            Square = mybir.ActivationFunctionType.Square
            for s in range(QS):
                t = acc[s]
                if not triv_bias:
                    nc.vector.tensor_add(t[:], t[:], bia[:])
                musum = lp.tile([P, 1], f32, tag="musum")
                nc.vector.reduce_sum(musum[:], t[:], axis=AX)
                # LN is scale-invariant: center as D*t - sum(t), compensate in
                # the sqrt (scale 1/D, bias D^2*eps) -- one op fewer per slice
                xc = lp.tile([P, D], f32, tag="xc")
                nc.vector.tensor_scalar(out=xc[:], in0=t[:], scalar1=float(D),
                                        scalar2=musum[:],
                                        op0=MUL, op1=mybir.AluOpType.subtract)
                sq = lp.tile([P, D], f32, tag="sq")
                vs = lp.tile([P, 1], f32, tag="vs")
                nc.scalar.activation(sq[:], xc[:], Square, accum_out=vs[:])
                sd = lp.tile([P, 1], f32, tag="sd")
                nc.scalar.activation(sd[:], vs[:], Sqrt, bias=eps2[:], scale=inv_d)
                rs = lp.tile([P, 1], f32, tag="rs")
                nc.vector.reciprocal(rs[:], sd[:])
                og = lp.tile([P, D], f32, tag="og")
                if triv_gamma:
                    nc.vector.tensor_scalar_mul(og[:], xc[:], rs[:])
                else:
                    nc.vector.scalar_tensor_tensor(og[:], in0=xc[:],
                                                   scalar=rs[:], in1=gam[:],
                                                   op0=MUL, op1=MUL)
                if triv_beta:
                    nc.sync.dma_start(out=out_d[s * P:(s + 1) * P, :], in_=og[:])
                else:
                    oo = lp.tile([P, D], f32, tag="oo")
                    nc.vector.tensor_add(oo[:], og[:], bet[:])
                    nc.sync.dma_start(out=out_d[s * P:(s + 1) * P, :], in_=oo[:])

    nc.compile()
    return nc


def _prep_host(inputs, N, QW):
    """Host-side input resharding: transposes, folded weights, mask offsets."""
    x = np.ascontiguousarray(np.asarray(inputs["x"], dtype=np.float32))
    ei = np.asarray(inputs["edge_index"]).astype(np.int64)
    Wq = np.asarray(inputs["Wq"], dtype=np.float64)
    Wk = np.asarray(inputs["Wk"], dtype=np.float64)
    Wv = np.asarray(inputs["Wv"], dtype=np.float64)
    Wo = np.asarray(inputs["Wo"], dtype=np.float64)
    Wp = np.asarray(inputs["Wp"], dtype=np.float64)
    bq = np.asarray(inputs["bq"], dtype=np.float64)
    bk = np.asarray(inputs["bk"], dtype=np.float64)
    bv = np.asarray(inputs["bv"], dtype=np.float64)
    bo = np.asarray(inputs["bo"], dtype=np.float64)
    bp = np.asarray(inputs["bp"], dtype=np.float64)
    gamma = np.asarray(inputs["gamma"], dtype=np.float32)
    beta = np.asarray(inputs["beta"], dtype=np.float32)

    assert not bq.any() and not bk.any(), \
        "nonzero q/k biases not wired in the device graph"

    xT = np.ascontiguousarray(x.T)                       # [D, N]
    wq_h = np.ascontiguousarray(
        np.stack([Wq[h].T for h in range(H)]).astype(np.float32))
    wk_h = np.ascontiguousarray(
        np.stack([Wk[h].T for h in range(H)]).astype(np.float32))
    # folded v' weight and total bias
    wv_l, bias_tot = [], bp.copy()
    for h in range(H):
        Wp_h = Wp[:, h * D:(h + 1) * D]                  # [f, e']
        G = Wo[h].T @ Wp_h.T                             # [e, f]
        wv_l.append(Wv[h].T @ G)                         # [d, f]
        bias_tot = bias_tot + bo[h] @ Wp_h.T + bv[h] @ G
    wv_h = np.ascontiguousarray(np.stack(wv_l).astype(np.float32))

    gam_b = np.ascontiguousarray(np.broadcast_to(gamma, (P, D)).astype(np.float32))
    bet_b = np.ascontiguousarray(np.broadcast_to(beta, (P, D)).astype(np.float32))
    bia_b = np.ascontiguousarray(
        np.broadcast_to(bias_tot.astype(np.float32), (P, D)))

    # mask stripes per core, pre-arranged to the SBUF layout
    # mall[p, c*QW + q] = adjacency[c*P + p, q0 + q]  (kv-major, symmetric+diag)
    import ml_dtypes
    adj = np.zeros((N, N), dtype=np.uint8)
    r, c = ei[0], ei[1]
    adj[r, c] = 1
    adj[c, r] = 1
    adj[np.arange(N), np.arange(N)] = 1
    KV = N // P
    malls = []
    for core in range(N_CORES):
        q0 = core * QW
        stripe = adj[:, q0:q0 + QW]                      # [N(kv), QW]
        m = stripe.reshape(KV, P, QW).transpose(1, 0, 2).reshape(P, KV * QW)
        malls.append(np.ascontiguousarray(m.astype(ml_dtypes.bfloat16)))
    return xT, wq_h, wk_h, wv_h, gam_b, bet_b, bia_b, malls


def _run(inputs, trace=False, mask_dt_name="bfloat16", mode="f32r",
         tmpdir=None):
    from concourse.bass_utils import run_bass_kernel_spmd
    from concourse.bass_interp import get_hw_module

    N = int(np.asarray(inputs["x"]).shape[0])
    QW = N // N_CORES
    (xT, wq_h, wk_h, wv_h, gam_b, bet_b, bia_b, malls) = \
        _prep_host(inputs, N, QW)

    if mode == "bf16":
        import ml_dtypes
        hdt = ml_dtypes.bfloat16
        xT = xT.astype(hdt)
        wq_h, wk_h, wv_h = (a.astype(hdt) for a in (wq_h, wk_h, wv_h))
    elif mode == "f32r":
        # fp32r operands must be pre-rounded (RNE dropping 12 mantissa bits);
        # matches walrus fp32_to_fp32r.
        def _r(a):
            b = a.view(np.uint32).astype(np.uint64)
            rb = (b + 0x7FF + ((b >> 12) & 1)) & np.uint64(0xFFFFF000)
            return rb.astype(np.uint32).view(np.float32)
        xT = _r(xT)
        wq_h, wk_h, wv_h = _r(wq_h), _r(wk_h), _r(wv_h)
    gamma = np.asarray(inputs["gamma"], np.float64)
    beta = np.asarray(inputs["beta"], np.float64)
    key = (N, QW, mask_dt_name, mode, not np.any(bia_b),
           bool((gamma == 1).all()), not beta.any())
    nc = _BUILD_CACHE.get(key)
    if nc is None:
        nc = _build(N, QW, mask_dt_name=mask_dt_name, mode=mode,
                    triv_bias=key[4], triv_gamma=key[5], triv_beta=key[6])
        _BUILD_CACHE[key] = nc
    old = nc.m
    nc.m = get_hw_module(nc.m)
    try:
        in_maps = []
        for core in range(N_CORES):
            q0 = core * QW
            in_maps.append({
                "xT": xT,
                "xq": np.ascontiguousarray(xT[:, q0:q0 + QW]),
                "wq": wq_h, "wk": wk_h, "wv": wv_h,
                "gamma_b": gam_b, "beta_b": bet_b, "bias_b": bia_b,
                "mall": malls[core],
            })
        res = run_bass_kernel_spmd(nc, in_maps, core_ids=list(range(N_CORES)),
                                   trace=trace, tmpdir=tmpdir)
    finally:
        nc.m = old
    out = np.concatenate([res.results[i]["out"] for i in range(N_CORES)], axis=0)
    return out.astype(np.float32), res


def kernel(**inputs) -> np.ndarray:
    out, _ = _run(inputs)
    return out


def _build2(N, mask_dt_name="bfloat16", mode="bf16"):
    """Hybrid sharding: core c owns head (c>>1) and row-half (c&1).

    Projections are per-head only (1/4 the replicated work of _build); the
    per-head partial outputs are summed across the 4 cores of each half via
    ReduceScatter (groups [[0,2,4,6],[1,3,5,7]]), one RS per 512-row
    superblock so all but the last overlap attention compute.  Core c's
    RS shard sb covers global rows (c&1)*2048 + sb*512 + (c>>1)*128.
    """
    import concourse.bacc as bacc
    import concourse.tile as tile
    import concourse.bass as bass
    from concourse import mybir

    f32 = mybir.dt.float32
    mask_dt = getattr(mybir.dt, mask_dt_name)
    cdt = {"f32r": mybir.dt.float32r, "bf16": mybir.dt.bfloat16,
           "f32": f32}[mode]
    Exp = mybir.ActivationFunctionType.Exp
    Copy = mybir.ActivationFunctionType.Copy
    Sqrt = mybir.ActivationFunctionType.Sqrt
    AX = mybir.AxisListType.X
    MUL = mybir.AluOpType.mult
    KV = N // P                 # kv chunks of 128
    HW_ = N // 2                # half-window width (2048)
    SBW = 512                   # superblock width
    NSB = HW_ // SBW            # superblocks (4)
    NB = N // 512
    D1 = D + 2

    nc = bacc.Bacc("TRN2", target_bir_lowering=False, debug=False,
                   num_devices=N_CORES)

    xT_d = nc.dram_tensor("xT", [D, N], cdt, kind="ExternalInput").ap()
    xq_d = nc.dram_tensor("xq", [D, HW_], cdt, kind="ExternalInput").ap()
    wq_d = nc.dram_tensor("wq", [D, D], cdt, kind="ExternalInput").ap()
    wk_d = nc.dram_tensor("wk", [D, D], cdt, kind="ExternalInput").ap()
    wv_d = nc.dram_tensor("wv", [D, D], cdt, kind="ExternalInput").ap()
    gam_d = nc.dram_tensor("gamma_b", [P, D], f32, kind="ExternalInput").ap()
    bet_d = nc.dram_tensor("beta_b", [P, D], f32, kind="ExternalInput").ap()
    bia_d = nc.dram_tensor("bias_b", [P, D], f32, kind="ExternalInput").ap()
    mal_d = nc.dram_tensor("mall", [P, NSB * KV * SBW], mask_dt,
                           kind="ExternalInput").ap()
    out_d = nc.dram_tensor("out", [NSB * P, D], f32, kind="ExternalOutput").ap()

    groups = [[0, 2, 4, 6], [1, 3, 5, 7]]

    with tile.TileContext(nc) as tc:
        with (
            tc.tile_pool(name="const", bufs=1) as cp,
            tc.tile_pool(name="maskp", bufs=2) as mp,
            tc.tile_pool(name="work", bufs=4) as wp,
            tc.tile_pool(name="accs", bufs=1) as ac,
            tc.tile_pool(name="ln", bufs=2) as lp,
            tc.tile_pool(name="psA", bufs=3, space="PSUM") as psA,
            tc.tile_pool(name="psO", bufs=1, space="PSUM") as psO,
            tc.tile_pool(name="dram", bufs=1, space="DRAM") as dp,
        ):
            wu = cp.tile([P, 640], mybir.dt.bfloat16, tag="wu")
            nc.vector.memset(wu[:], 0.125)
            wups = psA.tile([P, 512], f32, tag="ps", name="wups")
            for _ in range(16):
                nc.tensor.matmul(wups[:], lhsT=wu[:, :P], rhs=wu[:, P:P + 512],
                                 start=True, stop=True)

            xq = cp.tile([P, 2 * HW_], cdt, tag="xq")
            nc.sync.dma_start(out=xq[:].rearrange("p (i q) -> p i q", q=HW_),
                              in_=xq_d[:].rearrange("(i p) q -> p i q", p=P))
            wq = cp.tile([P, 2 * D], cdt, tag="wq")
            wk = cp.tile([P, 2 * D], cdt, tag="wk")
            wv = cp.tile([P, 2 * D], cdt, tag="wv")
            for wsb, wd in ((wq, wq_d), (wk, wk_d), (wv, wv_d)):
                nc.sync.dma_start(
                    out=wsb[:].rearrange("p (i d) -> p i d", i=2),
                    in_=wd[:].rearrange("(i p) d -> p i d", p=P))
            xT = cp.tile([P, 2 * N], cdt, tag="xT")
            nc.sync.dma_start(out=xT[:].rearrange("p (i n) -> p i n", n=N),
                              in_=xT_d[:].rearrange("(i p) n -> p i n", p=P))
            gam = cp.tile([P, D], f32, tag="gam")
            bet = cp.tile([P, D], f32, tag="bet")
            bia = cp.tile([P, D], f32, tag="bia")
            nc.sync.dma_start(out=gam[:], in_=gam_d[:])
            nc.sync.dma_start(out=bet[:], in_=bet_d[:])
            nc.sync.dma_start(out=bia[:], in_=bia_d[:])
            epsc = cp.tile([P, 1], f32, tag="epsc")
            nc.gpsimd.memset(epsc[:], EPS)
            onescol = cp.tile([P, 2 * KV], f32, tag="onescol")
            nc.gpsimd.memset(onescol[:], 1.0)

            # projections (single head)
            qT = cp.tile([P, 2 * HW_], cdt, tag="qT")
            for j in range(2):
                for qb in range(HW_ // 512):
                    ps = psA.tile([P, 512], f32, tag="ps")
                    for i in range(2):
                        nc.tensor.matmul(
                            ps[:],
                            lhsT=wq[:, i * D + j * P:i * D + j * P + P],
                            rhs=xq[:, i * HW_ + qb * 512:i * HW_ + (qb + 1) * 512],
                            start=(i == 0), stop=(i == 1))
                    nc.vector.tensor_copy(
                        qT[:, j * HW_ + qb * 512:j * HW_ + (qb + 1) * 512], ps[:])
            kT = cp.tile([P, 2 * N], cdt, tag="kT")
            for j in range(2):
                for b in range(NB):
                    ps = psA.tile([P, 512], f32, tag="ps")
                    for i in range(2):
                        nc.tensor.matmul(
                            ps[:],
                            lhsT=wk[:, i * D + j * P:i * D + j * P + P],
                            rhs=xT[:, i * N + b * 512:i * N + (b + 1) * 512],
                            start=(i == 0), stop=(i == 1))
                    if b % 2 == 0:
                        nc.scalar.copy(
                            kT[:, j * N + b * 512:j * N + (b + 1) * 512], ps[:])
                    else:
                        nc.vector.tensor_copy(
                            kT[:, j * N + b * 512:j * N + (b + 1) * 512], ps[:])
            vE = cp.tile([P, KV * D1], cdt, tag="vE")
            for c in range(KV):
                ps = psA.tile([P, 512], f32, tag="ps")
                for i in range(2):
                    nc.tensor.matmul(
                        ps[:, :D],
                        lhsT=xT[:, i * N + c * P:i * N + c * P + P],
                        rhs=wv[:, i * D:(i + 1) * D],
                        start=(i == 0), stop=(i == 1))
                nc.vector.tensor_copy(vE[:, c * D1:c * D1 + D], ps[:, :D])
            nc.vector.tensor_copy(
                vE[:].rearrange("p (c e) -> p c e", e=D1)[:, :, D:D + 2],
                onescol[:].rearrange("p (c e) -> p c e", e=2))

            # RS bounce buffers (one collective at the end)
            bounce_in = dp.tile([NSB * 4 * P, D], f32, name="bin")
            bounce_out = dp.tile([NSB * P, D], f32, name="bout")

            for sb in range(NSB):
                Msb = mp.tile([P, KV * SBW], mask_dt, tag="Msb",
                              name=f"Msb{sb}")
                for c in range(KV):
                    nc.sync.dma_start(
                        out=Msb[:, c * SBW:(c + 1) * SBW],
                        in_=mal_d[:, (sb * KV + c) * SBW:(sb * KV + c + 1) * SBW])
                oPS = [psO.tile([P, D1], f32, tag=f"oPS{s}", name=f"oPS{sb}_{s}")
                       for s in range(4)]

                def o_mms(c, et):
                    for s in range(4):
                        nc.tensor.matmul(oPS[s][:],
                                         lhsT=et[:, s * P:(s + 1) * P],
                                         rhs=vE[:, c * D1:(c + 1) * D1],
                                         start=(c == 0), stop=(c == KV - 1))

                prev = None
                for c in range(KV):
                    sc = psA.tile([P, 512], f32, tag="ps", name=f"sc{sb}_{c}")
                    for j in range(2):
                        nc.tensor.matmul(
                            sc[:],
                            lhsT=kT[:, j * N + c * P:j * N + c * P + P],
                            rhs=qT[:, j * HW_ + sb * SBW:j * HW_ + (sb + 1) * SBW],
                            start=(j == 0), stop=(j == 1))
                    et = wp.tile([P, SBW], cdt, tag="et", name=f"et{sb}_{c}")
                    nc.scalar.activation(et[:], sc[:], Exp, scale=1.0 / 16.0)
                    nc.vector.tensor_mul(et[:], et[:],
                                         Msb[:, c * SBW:(c + 1) * SBW])
                    if prev is not None:
                        o_mms(*prev)
                    prev = (c, et)
                o_mms(*prev)

                for s in range(4):
                    rec = lp.tile([P, 1], f32, tag="rec", name=f"rec{sb}_{s}")
                    nc.vector.reciprocal(rec[:], oPS[s][:, D:D + 1])
                    par = lp.tile([P, D], f32, tag="par", name=f"par{sb}_{s}")
                    nc.vector.tensor_scalar_mul(par[:], oPS[s][:, 0:D], rec[:])
                    nc.sync.dma_start(
                        out=bounce_in[(sb * 4 + s) * P:(sb * 4 + s + 1) * P, :],
                        in_=par[:])
            nc.gpsimd.collective_compute(
                "ReduceScatter", mybir.AluOpType.add,
                replica_groups=groups,
                ins=[bounce_in[:].opt()],
                outs=[bounce_out[:].opt()],
            )

            # post-RS: bias + LayerNorm + store, per superblock shard
            inv_d = 1.0 / D
            for sb in range(NSB):
                t = ac.tile([P, D], f32, tag="acc", name=f"post{sb}")
                nc.sync.dma_start(out=t[:],
                                  in_=bounce_out[sb * P:(sb + 1) * P, :])
                nc.vector.tensor_add(t[:], t[:], bia[:])
                musum = lp.tile([P, 1], f32, tag="musum", name=f"mus{sb}")
                nc.vector.reduce_sum(musum[:], t[:], axis=AX)
                mu = lp.tile([P, 1], f32, tag="mu", name=f"mu{sb}")
                nc.scalar.activation(mu[:], musum[:], Copy, scale=inv_d)
                xc = lp.tile([P, D], f32, tag="xc", name=f"xc{sb}")
                nc.vector.tensor_scalar_sub(xc[:], t[:], mu[:])
                sq = lp.tile([P, D], f32, tag="sq", name=f"sq{sb}")
                nc.vector.tensor_mul(sq[:], xc[:], xc[:])
                vs = lp.tile([P, 1], f32, tag="vs", name=f"vs{sb}")
                nc.vector.reduce_sum(vs[:], sq[:], axis=AX)
                sd = lp.tile([P, 1], f32, tag="sd", name=f"sd{sb}")
                nc.scalar.activation(sd[:], vs[:], Sqrt, bias=epsc[:],
                                     scale=inv_d)
                rs = lp.tile([P, 1], f32, tag="rs", name=f"rs{sb}")
                nc.vector.reciprocal(rs[:], sd[:])
                og = lp.tile([P, D], f32, tag="og", name=f"og{sb}")
                nc.vector.scalar_tensor_tensor(og[:], in0=xc[:], scalar=rs[:],
                                               in1=gam[:], op0=MUL, op1=MUL)
                oo = lp.tile([P, D], f32, tag="oo", name=f"oo{sb}")
                nc.vector.tensor_add(oo[:], og[:], bet[:])
                nc.sync.dma_start(out=out_d[sb * P:(sb + 1) * P, :], in_=oo[:])

    nc.compile()
    return nc


def _run2(inputs, trace=False, mask_dt_name="bfloat16", mode="bf16",
          tmpdir=None):
    from concourse.bass_utils import run_bass_kernel_spmd
    from concourse.bass_interp import get_hw_module
    import ml_dtypes

    N = int(np.asarray(inputs["x"]).shape[0])
    QW = N // N_CORES
    (xT, wq_h, wk_h, wv_h, gam_b, bet_b, bia_b, _malls) =         _prep_host(inputs, N, QW)
    ei = np.asarray(inputs["edge_index"]).astype(np.int64)
    adj = np.zeros((N, N), dtype=np.uint8)
    adj[ei[0], ei[1]] = 1
    adj[ei[1], ei[0]] = 1
    adj[np.arange(N), np.arange(N)] = 1

    if mode == "bf16":
        hdt = ml_dtypes.bfloat16
        xT = xT.astype(hdt)
        wq_h, wk_h, wv_h = (a.astype(hdt) for a in (wq_h, wk_h, wv_h))

    KV = N // P
    HW_ = N // 2
    SBW = 512
    NSB = HW_ // SBW
    nc = _build2(N, mask_dt_name=mask_dt_name, mode=mode)
    old = nc.m
    nc.m = get_hw_module(nc.m)
    try:
        in_maps = []
        for core in range(N_CORES):
            h, r = core >> 1, core & 1
            q0 = r * HW_
            stripe = adj[:, q0:q0 + HW_]
            m = np.zeros((P, NSB * KV * SBW), np.uint8)
            for sb in range(NSB):
                blk = stripe[:, sb * SBW:(sb + 1) * SBW]
                m[:, sb * KV * SBW:(sb + 1) * KV * SBW] = (
                    blk.reshape(KV, P, SBW).transpose(1, 0, 2)
                    .reshape(P, KV * SBW))
            in_maps.append({
                "xT": xT,
                "xq": np.ascontiguousarray(xT[:, q0:q0 + HW_]),
                "wq": wq_h[h], "wk": wk_h[h], "wv": wv_h[h],
                "gamma_b": gam_b, "beta_b": bet_b, "bias_b": bia_b,
                "mall": np.ascontiguousarray(m.astype(ml_dtypes.bfloat16)),
            })
        res = run_bass_kernel_spmd(nc, in_maps, core_ids=list(range(N_CORES)),
                                   trace=trace, tmpdir=tmpdir)
    finally:
        nc.m = old
    out = np.zeros((N, D), np.float32)
    for core in range(N_CORES):
        h, r = core >> 1, core & 1
        g0 = r * HW_ + h * (NSB * P)
        out[g0:g0 + NSB * P] = res.results[core]["out"]
    return out.astype(np.float32), res
